# revision 13
# baseline (speedup 1.0000x reference)
"""Trainium2 Bass kernel for DigitConvolutionalModel.

Computes: out = relu(conv2d_valid(x.reshape(B,28,28), w3x3)).reshape(B,676) @ W + b

Strategy (pure data parallel over 8 NeuronCores, 8192 images/core), bf16:
  - Host: pack x per core into 16 blocks of [120 part, 7 chunks, 512 batch]
    bf16, fully contiguous per block (one 860KB DMA per block). Chunk c =
    image rows 4c..4c+3. Even chunks store rows [0,1,2,3] at partitions
    0..111; odd chunks store rows [2,3] at partitions 0..55 and rows [0,1]
    at partitions 64..119 (so B-phase matmul pairs land on disjoint PE row
    strips and run concurrently).
  - Device per 512-image block (13 PE pass-equivalents of N=512):
      A-phase: 7 matmuls (stationary CAe/CAo [120,128], K=120) -> 7 PSUM
               banks, one per 4-output-row conv group.
      B-phase: 6 matmuls (stationary CB [56,104], K=56) issued as 3
               concurrent row-tiled pairs (tile_position (64,0)/(0,0)),
               closing groups 0..5.
      ReLU PSUM->SBUF bf16 (ACT: groups 0,2,4,6; DVE: 1,3,5).
      FC: 7 matmuls (W chunks [104,32], M=32) col-tiled over 4 strips
          (tile_position (0,32j)) -> 2 rounds into one PSUM bank;
          DVE copy -> SBUF; Sel matmul [128,10] reduces the 4 partials.
          FC for block j is emitted after conv of block j+1 (software
          pipelining) so the PE never waits on the ReLU copies.
      Bias-add via DVE -> outT store on the ACT HWDGE ring.
  - Host: gather per-core outT [10, 8192] and transpose into out[B, 10].
"""

import os

import numpy as np
import ml_dtypes

import concourse.bass as bass
import concourse.mybir as mybir
import concourse.tile as tile
from concourse import bacc
from concourse.bass import ts
from concourse.bass_utils import run_bass_kernel_spmd

BF16 = ml_dtypes.bfloat16
PRECISION = "bf16"

# Problem geometry (fixed by the task spec)
B_FULL = 65536
IMG = 28
KW = 3
OH = IMG - KW + 1          # 26
NPIX = IMG * IMG           # 784
NFEAT = OH * OH            # 676
NOUT = 10

N_CORES = 8
B_CORE = B_FULL // N_CORES  # 8192
NB = 512                    # images per block
N_BLOCKS = B_CORE // NB     # 16

G_ROWS = 4
G_FEAT = G_ROWS * OH        # 104
N_GROUPS = 7                # 6 groups of 4 out-rows + 1 group of 2 (52 feats)
CHUNK_ROWS = 4
XPART = 120                 # partitions used by the packed x layout
MPAD = 128                  # stationary columns padded to 128 (FWL)
FCM = 32                    # FC stationary columns (one 32-col strip)

WARM_MMS = 12               # HAM warm-up matmuls (N=512) before the first
                            # block: open the clock gate (~3.4us busy) and
                            # cover the first x block's arrival (~5us now
                            # that it is split across both HWDGE rings)


def _feat_count(t: int) -> int:
    return min(G_FEAT, NFEAT - G_FEAT * t)


def _part_base(c: int, r: int) -> int:
    """Partition base of image-row r (0..3) of chunk c in the packed layout."""
    if c % 2 == 0:
        return r * IMG
    return (r - 2) * IMG if r >= 2 else 64 + r * IMG


def build_conv_mats(conv_w: np.ndarray):
    """CA[pixel, feat] (within 4-row chunk) and CB[pixel, feat] (2-row head
    of the next chunk) express the 3x3 valid conv for one 4-out-row group."""
    w = np.asarray(conv_w, np.float32)
    CA = np.zeros((CHUNK_ROWS * IMG, G_FEAT), np.float32)
    CB = np.zeros((2 * IMG, G_FEAT), np.float32)
    for ol in range(G_ROWS):
        for oj in range(OH):
            m = ol * OH + oj
            for di in range(KW):
                for dj in range(KW):
                    r = ol + di
                    c = oj + dj
                    if r < CHUNK_ROWS:
                        CA[r * IMG + c, m] = w[di, dj]
                    else:
                        CB[(r - CHUNK_ROWS) * IMG + c, m] = w[di, dj]
    return CA, CB


def build_selector() -> np.ndarray:
    """S[32j + o, o] = 1: sums the 4 col-strip FC partials."""
    S = np.zeros((128, NOUT), np.float32)
    for j in range(4):
        for o in range(NOUT):
            S[32 * j + o, o] = 1.0
    return S


def build_program():
    f32 = mybir.dt.float32
    bf = mybir.dt.bfloat16

    nc = bacc.Bacc()
    # Partition-major across blocks: per-partition bytes for a k-block
    # super-chunk DMA are contiguous (k*7168 B descriptors amortize the
    # ~190ns per-descriptor overhead that capped the ring at ~16 GB/s/eng).
    xP = nc.declare_dram_parameter("xP", [XPART, N_BLOCKS, N_GROUPS, NB], bf,
                                   isOutput=False)
    cae_d = nc.declare_dram_parameter("CAe", [XPART, MPAD], bf, isOutput=False)
    cao_d = nc.declare_dram_parameter("CAo", [XPART, MPAD], bf, isOutput=False)
    cb_d = nc.declare_dram_parameter("CB2", [XPART, G_FEAT], bf, isOutput=False)
    wp_d = nc.declare_dram_parameter("Wp", [G_FEAT, N_GROUPS, FCM], bf,
                                     isOutput=False)
    sel_d = nc.declare_dram_parameter("Sel", [128, NOUT], bf, isOutput=False)
    bias_d = nc.declare_dram_parameter("bias", [NOUT, 1], f32, isOutput=False)
    outT = nc.declare_dram_parameter("outT", [NOUT, N_BLOCKS, NB], f32,
                                     isOutput=True)

    Relu = mybir.ActivationFunctionType.Relu
    Ident = mybir.ActivationFunctionType.Identity

    # x super-chunk sizes: small first (fast pipeline start), large later
    # (28KB descriptors reach peak per-engine DMA throughput).
    CHUNKS = (1, 1, 2, 4, 4, 4)
    assert sum(CHUNKS) == N_BLOCKS

    with tile.TileContext(nc) as tc:
        with (
            tc.tile_pool(name="const", bufs=1) as const,
            tc.tile_pool(name="feat", bufs=14) as fpool,
            tc.tile_pool(name="fcsb", bufs=2) as fcpool,
            tc.tile_pool(name="osb", bufs=3) as opool,
            tc.tile_pool(name="psc", bufs=7, space="PSUM") as psc,
            tc.tile_pool(name="pso", bufs=1, space="PSUM") as pso,
        ):
            # Constants first on the sync ring: ~130KB total, lands in ~2us,
            # before the first conv matmul needs them.
            cae = const.tile([XPART, MPAD], bf)
            nc.sync.dma_start(out=cae[:, :], in_=cae_d[:, :])
            cao = const.tile([XPART, MPAD], bf)
            nc.sync.dma_start(out=cao[:, :], in_=cao_d[:, :])
            cb = const.tile([XPART, G_FEAT], bf)
            nc.sync.dma_start(out=cb[:, :], in_=cb_d[:, :])
            wp = const.tile([G_FEAT, N_GROUPS, FCM], bf)
            nc.sync.dma_start(out=wp[:, :, :], in_=wp_d[:, :, :])
            sel = const.tile([128, NOUT], bf)
            nc.sync.dma_start(out=sel[:, :], in_=sel_d[:, :])
            bias_sb = const.tile([NOUT, 1], f32)
            nc.sync.dma_start(out=bias_sb[:, :], in_=bias_d[:, :])

            # x super-chunk loads, alternating HWDGE rings. All of x stays
            # resident in SBUF (~115KB/partition, fits comfortably).
            blk2tile = {}
            b0 = 0
            for ci, nb in enumerate(CHUNKS):
                xt = const.tile([XPART, nb, N_GROUPS, NB], bf)
                eng = nc.sync if ci % 2 == 0 else nc.scalar
                eng.dma_start(
                    out=xt[:, :, :, :],
                    in_=xP[:, b0:b0 + nb, :, :],
                )
                for b in range(nb):
                    blk2tile[b0 + b] = (xt, b)
                b0 += nb

            # PE warm-up during the first x DMA (open the HAM clock gate).
            # The source tile is memset on GpSimd (no DMA dependency) so the
            # warm-up starts right after the engine preamble.
            wsrc = const.tile([MPAD, NB], bf)
            nc.gpsimd.memset(wsrc[:, :], 0)
            warm_ps = psc.tile([MPAD, NB], mybir.dt.float32, tag="convps")
            for _ in range(WARM_MMS):
                nc.tensor.matmul(
                    warm_ps[:, :], wsrc[:, :MPAD], wsrc[:, :],
                    start=True, stop=True,
                )

            state = {}

            def emit_conv(j, xc, a_order=(0, 2, 4, 6, 1, 3, 5)):
                # A-phase: even chunks (CAe) then odd chunks (CAo) so the
                # stationary operand changes only once. xc(t) -> chunk AP.
                pss = [None] * N_GROUPS
                for t in a_order:
                    ps = psc.tile([MPAD, NB], mybir.dt.float32, tag="convps")
                    ca = cae if t % 2 == 0 else cao
                    nc.tensor.matmul(
                        ps[:, :], ca[:, :], xc(t),
                        start=True, stop=(t == 6),
                    )
                    pss[t] = ps
                # B-phase: 3 row-tiled concurrent pairs.
                # B(t) reads chunk t+1 rows 0,1: odd chunk -> partitions
                # 64..119 (row strips 2,3); even chunk -> 0..55 (strips 0,1).
                for t in range(N_GROUPS - 1):
                    if t % 2 == 0:  # chunk t+1 odd: hi placement
                        nc.tensor.matmul(
                            pss[t][:G_FEAT, :], cb[64:120, :],
                            xc(t + 1)[64:120, :],
                            start=False, stop=True, tile_position=(64, 0),
                        )
                    else:           # chunk t+1 even: lo placement
                        nc.tensor.matmul(
                            pss[t][:G_FEAT, :], cb[0:56, :],
                            xc(t + 1)[0:56, :],
                            start=False, stop=True, tile_position=(0, 0),
                        )
                # ReLU PSUM->SBUF bf16 (ACT: 0,2,4,6; DVE: 1,3,5).
                feats = []
                for t in range(N_GROUPS):
                    nf = _feat_count(t)
                    ft = fpool.tile([nf, NB], bf, tag="feat")
                    if t % 2 == 0:
                        nc.scalar.activation(ft[:, :], pss[t][:nf, :], Relu)
                    else:
                        nc.vector.tensor_scalar_max(ft[:, :], pss[t][:nf, :], 0.0)
                    feats.append(ft)
                state[j] = feats

            def emit_fc(j):
                feats = state.pop(j)
                # 7 col-tiled matmuls into one PSUM bank: round 1 strips
                # 0..3 (each clears its strip), round 2 strips 0..2 accum.
                ops = pso.tile([128, NB], mybir.dt.float32, tag="outps")
                for t in range(N_GROUPS):
                    nf = _feat_count(t)
                    strip = 32 * (t % 4)
                    nc.tensor.matmul(
                        ops[strip:strip + FCM, :], wp[:nf, t, :],
                        feats[t][:, :],
                        start=(t < 4), stop=(t >= 3),
                        tile_position=(0, strip), skip_group_check=True,
                    )
                fcsb = fcpool.tile([128, NB], bf, tag="fcsb")
                nc.vector.tensor_copy(fcsb[:, :], ops[:, :])
                # Sel output reuses the fc-partial bank (already copied out).
                nc.tensor.matmul(
                    ops[:NOUT, :], sel[:, :], fcsb[:, :], start=True, stop=True,
                    skip_group_check=True,
                )
                osb = opool.tile([NOUT, NB], f32, tag="osb")
                nc.vector.tensor_scalar(
                    osb[:, :], ops[:NOUT, :], bias_sb[:, :], None,
                    op0=mybir.AluOpType.add,
                )
                eng = nc.scalar if j % 2 == 0 else nc.sync
                eng.dma_start(out=outT[:, j, :], in_=osb[:, :])

            for j in range(N_BLOCKS):
                xt, b = blk2tile[j]
                emit_conv(j, lambda t, xt=xt, b=b: xt[:, b, t, :])
                if j >= 1:
                    emit_fc(j - 1)
            emit_fc(N_BLOCKS - 1)

    nc.finalize()
    return nc


def prepare_inputs(x, conv_w, W, b):
    CA, CB = build_conv_mats(conv_w)

    # Stationary conv matrices in the packed-partition layouts.
    CAe = np.zeros((XPART, MPAD), np.float32)
    CAe[: CHUNK_ROWS * IMG, :G_FEAT] = CA
    CAo = np.zeros((XPART, MPAD), np.float32)
    for r in range(CHUNK_ROWS):
        base = _part_base(1, r)
        CAo[base:base + IMG, :G_FEAT] = CA[r * IMG:(r + 1) * IMG, :]
    CB2 = np.zeros((XPART, G_FEAT), np.float32)
    CB2[0:2 * IMG, :] = CB       # lo placement (next chunk even)
    CB2[64:64 + 2 * IMG, :] = CB  # hi placement (next chunk odd)

    Wf = np.asarray(W, np.float32)
    Wp = np.zeros((G_FEAT, N_GROUPS, FCM), np.float32)
    for t in range(N_GROUPS):
        nf = _feat_count(t)
        Wp[:nf, t, :NOUT] = Wf[G_FEAT * t: G_FEAT * t + nf, :]
    Sel = build_selector()
    bias = np.asarray(b, np.float32).reshape(NOUT, 1)

    CAe, CAo, CB2, Wp, Sel = (a.astype(BF16) for a in (CAe, CAo, CB2, Wp, Sel))

    # Pack x: [B, 784] -> per core [120, N_BLOCKS, 7, NB] bf16
    # (partition-major across blocks for contiguous super-chunk DMAs).
    xbf = np.asarray(x, np.float32).astype(BF16)
    # [core, block, b, row, col] view of the batch-major input
    xv = xbf.reshape(N_CORES, N_BLOCKS, NB, IMG, IMG)
    in_maps = []
    for core in range(N_CORES):
        xp = np.zeros((XPART, N_BLOCKS, N_GROUPS, NB), BF16)
        for c in range(N_GROUPS):
            for r in range(CHUNK_ROWS):
                base = _part_base(c, r)
                # [col, block, b] <- [block, b, col]
                xp[base:base + IMG, :, c, :] = (
                    xv[core, :, :, 4 * c + r, :].transpose(2, 0, 1)
                )
        in_maps.append(
            {
                "xP": xp,
                "CAe": CAe,
                "CAo": CAo,
                "CB2": CB2,
                "Wp": Wp,
                "Sel": Sel,
                "bias": bias,
            }
        )
    return in_maps


def _enable_ldw_opt():
    """Let walrus dedup repeated LDWEIGHTS so same-stationary matmul runs
    pipeline back-to-back instead of paying an LDW + drain per matmul."""
    import concourse.bass_utils as bu

    if getattr(bu, "_ldw_opt_patched", False):
        return
    orig = bu.run_command

    def patched(argv, **kw):
        argv = [
            "--enable-ldw-opt=true" if a == "--enable-ldw-opt=false" else a
            for a in argv
        ]
        return orig(argv, **kw)

    bu.run_command = patched
    bu._ldw_opt_patched = True


def run(x, conv_w, W, b, trace=False, **spmd_kwargs):
    if os.environ.get("KERNEL_LDW_OPT") == "1":
        _enable_ldw_opt()
    in_maps = prepare_inputs(x, conv_w, W, b)
    nc = build_program()
    res = run_bass_kernel_spmd(
        nc, in_maps, list(range(N_CORES)), trace=trace, **spmd_kwargs
    )
    out = np.empty((B_FULL, NOUT), np.float32)
    for c in range(N_CORES):
        out[c * B_CORE:(c + 1) * B_CORE, :] = (
            res.results[c]["outT"].reshape(NOUT, B_CORE).T
        )
    return out, res


def kernel(x, conv_w, W, b):
    out, _ = run(x, conv_w, W, b, trace=False)
    return out



# revision 16
# speedup vs baseline: 1.3534x; 1.3534x over previous
"""Trainium2 Bass kernel for DigitConvolutionalModel.

Computes: out = relu(conv2d_valid(x.reshape(B,28,28), w3x3)).reshape(B,676) @ W + b

Strategy (pure data parallel over 8 NeuronCores, 8192 images/core), bf16:
  - Host: pack x per core partition-major [120, 16 blocks, 7, 512] bf16.
    Pass-column c holds 2-row chunk c (rows 2c,2c+1) at partitions 0..55
    and chunk c+7 (rows 2c+14,2c+15) at partitions 64..119.
  - x streams in 6 super-chunk DMAs (1,1,2,4,4,4 blocks) alternating the
    two HWDGE rings (sync/scalar); multi-block per-partition-contiguous
    descriptors (up to 28KB) amortize per-descriptor overhead. All of x
    stays resident in SBUF (~115KB/partition).
  - Conv per 512-image block: 7 passes. Pass p = 4 quadrant-tiled
    matmuls (2x2 tile_position grid, tile_size 64x64): chunk p lo rows x
    {A-taps -> group p, B-taps -> group p-1} and chunk p+7 hi rows x
    {A-taps -> group p+7, B-taps -> group p+6}. Groups g=0..12 are 2
    output rows (52 feats); group g accumulates A (chunk g) + B (chunk
    g+1) in PSUM partitions 64*(g%2)..+52 of pair bank g//2.
  - ReLU per pair bank [116,512] -> SBUF bf16 (gap partitions 52..63
    memset to 0 once per bank; FC weight rows there are 0).
  - FC: 7 matmuls (Wp chunks [116,32], col-tiled 4 strips) -> 2 rounds
    into one PSUM bank; DVE copy -> SBUF; Sel matmul [128,10] reduces
    the 4 col-strip partials; DVE bias-add; per-block outT store.
  - FC for block j emitted after conv of block j+1 (software pipelining).
"""

import numpy as np
import ml_dtypes

import concourse.bass as bass
import concourse.mybir as mybir
import concourse.tile as tile
from concourse import bacc
from concourse.bass_utils import run_bass_kernel_spmd

BF16 = ml_dtypes.bfloat16
PRECISION = "bf16"

# Problem geometry (fixed by the task spec)
B_FULL = 65536
IMG = 28
KW = 3
OH = IMG - KW + 1          # 26
NPIX = IMG * IMG           # 784
NFEAT = OH * OH            # 676
NOUT = 10

N_CORES = 8
B_CORE = B_FULL // N_CORES  # 8192
NB = 512                    # images per block
N_BLOCKS = B_CORE // NB     # 16

N_PASS = 7                  # conv passes per block (2 chunks each)
N_CHUNK = 14                # 2-row input chunks
N_GRP = 13                  # 2-out-row groups of 52 feats
G2 = 2 * OH                 # 52
XPART = 120                 # partitions used by the packed x layout
FCM = 32                    # FC stationary columns (one 32-col strip)
N_FC = 7                    # FC chunks (6 pairs of groups + 1 single)
KFC = 116                   # FC contraction for a group pair (52+12gap+52)

# Const blob column offsets (bf16, [128, BLOB_COLS])
OFF_SA = 0                  # [120, 52]  A-taps (lo rows 0..55, hi 64..119)
OFF_SB = 52                 # [120, 52]  B-taps
OFF_WP = 104                # [116, 7, 32] FC weights
OFF_SEL = 104 + 7 * FCM     # [128, 10]  col-strip reduction selector
BLOB_COLS = OFF_SEL + NOUT

WARM_MMS = 8                # HAM warm-up matmuls (open the clock gate
                            # during the first x super-chunk's transfer)

# x super-chunk sizes: small first (fast pipeline start), large later
# (28KB descriptors reach peak per-engine DMA throughput).
CHUNKS = (1, 1, 2, 4, 4, 4)
LOOKAHEAD = 6               # issue chunk DMA when first block <= j + LOOKAHEAD


def build_conv_mats(conv_w: np.ndarray):
    """SA[pixel, feat]: taps of a group's own chunk (input rows 2g,2g+1).
    SB[pixel, feat]: taps of the next chunk (rows 2g+2,2g+3). feat =
    26*l + oj for out row 2g+l, col oj."""
    w = np.asarray(conv_w, np.float32)
    SA = np.zeros((56, G2), np.float32)
    SB = np.zeros((56, G2), np.float32)
    for l in range(2):
        for oj in range(OH):
            f = OH * l + oj
            for r in range(2):
                for dj in range(KW):
                    c = oj + dj
                    diA = r - l
                    if 0 <= diA < KW:
                        SA[r * IMG + c, f] = w[diA, dj]
                    diB = 2 + r - l
                    if 0 <= diB < KW:
                        SB[r * IMG + c, f] = w[diB, dj]
    return SA, SB


def build_selector() -> np.ndarray:
    """S[32j + o, o] = 1: sums the 4 col-strip FC partials."""
    S = np.zeros((128, NOUT), np.float32)
    for j in range(4):
        for o in range(NOUT):
            S[FCM * j + o, o] = 1.0
    return S


def build_program():
    f32 = mybir.dt.float32
    bf = mybir.dt.bfloat16

    nc = bacc.Bacc()
    # Partition-major across blocks: per-partition bytes for a k-block
    # super-chunk DMA are contiguous (k*7168B descriptors amortize the
    # ~190ns per-descriptor overhead that caps small-descriptor DMAs).
    xP = nc.declare_dram_parameter("xP", [XPART, N_BLOCKS, N_PASS, NB], bf,
                                   isOutput=False)
    blob_d = nc.declare_dram_parameter("blob", [128, BLOB_COLS], bf,
                                       isOutput=False)
    bias_d = nc.declare_dram_parameter("bias", [NOUT, 1], f32, isOutput=False)
    outT = nc.declare_dram_parameter("outT", [NOUT, N_BLOCKS, NB], f32,
                                     isOutput=True)

    Relu = mybir.ActivationFunctionType.Relu

    # pair index of the relu emitted after each conv pass
    RELU_AFTER = {2: [0], 3: [4], 4: [1], 5: [5], 6: [2, 3, 6]}
    RELU_ON_ACT = {0, 1, 2, 6}  # pairs relu'd on ACT; rest on DVE

    with tile.TileContext(nc) as tc:
        with (
            tc.tile_pool(name="const", bufs=1) as const,
            tc.tile_pool(name="feat", bufs=14) as fpool,
            tc.tile_pool(name="fcsb", bufs=2) as fcpool,
            tc.tile_pool(name="osb", bufs=3) as opool,
            tc.tile_pool(name="psc", bufs=7, space="PSUM") as psc,
            tc.tile_pool(name="pso", bufs=1, space="PSUM") as pso,
        ):
            # One DMA for all bf16 constants (~150KB, lands ~2.5us), one
            # tiny SWDGE DMA for the f32 bias.
            blob = const.tile([128, BLOB_COLS], bf)
            nc.sync.dma_start(out=blob[:, :], in_=blob_d[:, :])
            bias_sb = const.tile([NOUT, 1], f32)
            nc.gpsimd.dma_start(out=bias_sb[:, :], in_=bias_d[:, :])

            def SA(rb):   # stationary A-taps at row base rb (0 or 64)
                return blob[rb:rb + 56, OFF_SA:OFF_SA + G2]

            def SB(rb):
                return blob[rb:rb + 56, OFF_SB:OFF_SB + G2]

            def WP(t, k):
                return blob[0:k, OFF_WP + FCM * t:OFF_WP + FCM * (t + 1)]

            sel = blob[:, OFF_SEL:OFF_SEL + NOUT]

            # PE warm-up during the first x super-chunk's transfer. Source
            # is DVE-memset (no DMA dependency) so warm-up starts right
            # after the engine preamble and opens the HAM clock gate.
            wsrc = const.tile([128, NB], bf)
            nc.vector.memset(wsrc[:, :], 0)
            warm_ps = psc.tile([128, NB], mybir.dt.float32, tag="convps")
            for _ in range(WARM_MMS):
                nc.tensor.matmul(
                    warm_ps[:, :], wsrc[:, :128], wsrc[:, :],
                    start=True, stop=True,
                )
            # Zero the psum gap partitions (52..63) of all 7 conv banks
            # once: ReLU reads [0:116] across the gap; FC weight rows
            # 52..63 are 0, but relu(NaN-garbage) would poison the FC.
            for _ in range(7):
                t = psc.tile([128, NB], mybir.dt.float32, tag="convps",
                             name="gapz")
                nc.vector.memset(t[:, :], 0)

            # x super-chunk DMAs are issued lazily inside the block loop
            # (program order near their consumers keeps the per-engine
            # instruction streams from serializing them behind compute).
            blk2tile = {}
            chunk_plan = []
            b0 = 0
            for ci, nb in enumerate(CHUNKS):
                chunk_plan.append((ci, b0, nb))
                b0 += nb
            next_chunk = [0]

            def issue_chunks(j):
                while next_chunk[0] < len(chunk_plan):
                    ci, b0, nb = chunk_plan[next_chunk[0]]
                    if b0 > j + LOOKAHEAD:
                        break
                    xt = const.tile([XPART, nb, N_PASS, NB], bf, name=f"xchunk{ci}")
                    eng = nc.sync if ci % 2 == 0 else nc.scalar
                    eng.dma_start(out=xt[:, :, :, :],
                                  in_=xP[:, b0:b0 + nb, :, :])
                    for b in range(nb):
                        blk2tile[b0 + b] = (xt, b)
                    next_chunk[0] += 1

            state = {}

            def emit_conv(j):
                xt, b = blk2tile[j]
                pairs = {}
                written = {}

                def pt(g):
                    k = g // 2
                    if k not in pairs:
                        pairs[k] = psc.tile([128, NB], mybir.dt.float32,
                                            tag="convps", name=f"pair{k}")
                    s = 64 * (g % 2)
                    return pairs[k][s:s + G2, :]

                def mm(g, stat, rb):
                    first = g not in written
                    written[g] = True
                    nc.tensor.matmul(
                        pt(g), stat(rb), xt[rb:rb + 56, b, p, :],
                        start=first, stop=not first,
                        tile_position=(rb, 64 * (g % 2)),
                        skip_group_check=True,
                    )

                feats = {}
                for p in range(N_PASS):
                    mm(p, SA, 0)                    # A: chunk p -> group p
                    if p >= 1:
                        mm(p - 1, SB, 0)            # B: chunk p -> group p-1
                    if p <= 5:
                        mm(p + 7, SA, 64)           # A: chunk p+7 -> group p+7
                    mm(p + 6, SB, 64)               # B: chunk p+7 -> group p+6
                    for k in RELU_AFTER.get(p, ()):
                        kf = KFC if k < N_FC - 1 else G2
                        ft = fpool.tile([kf, NB], bf, tag="feat", name=f"ft{k}")
                        if k in RELU_ON_ACT:
                            nc.scalar.activation(
                                ft[:, :], pairs[k][:kf, :], Relu)
                        else:
                            nc.vector.tensor_scalar_max(
                                ft[:, :], pairs[k][:kf, :], 0.0)
                        feats[k] = ft
                state[j] = feats

            def emit_fc(j):
                feats = state.pop(j)
                # 7 col-tiled matmuls into one PSUM bank: round 1 strips
                # 0..3 (each clears its strip), round 2 strips 0..2 accum.
                ops = pso.tile([128, NB], mybir.dt.float32, tag="outps")
                for t in range(N_FC):
                    kf = KFC if t < N_FC - 1 else G2
                    strip = FCM * (t % 4)
                    nc.tensor.matmul(
                        ops[strip:strip + FCM, :], WP(t, kf),
                        feats[t][:, :],
                        start=(t < 4), stop=(t >= 3),
                        tile_position=(0, strip), skip_group_check=True,
                    )
                fcsb = fcpool.tile([128, NB], bf, tag="fcsb")
                nc.vector.tensor_copy(fcsb[:, :], ops[:, :])
                # Sel output reuses the fc-partial bank (already copied out).
                nc.tensor.matmul(
                    ops[:NOUT, :], sel[:, :], fcsb[:, :], start=True,
                    stop=True, skip_group_check=True,
                )
                osb = opool.tile([NOUT, NB], f32, tag="osb")
                nc.vector.tensor_scalar(
                    osb[:, :], ops[:NOUT, :], bias_sb[:, :], None,
                    op0=mybir.AluOpType.add,
                )
                eng = nc.scalar if j % 2 == 0 else nc.sync
                eng.dma_start(out=outT[:, j, :], in_=osb[:, :])

            for j in range(N_BLOCKS):
                issue_chunks(j)
                emit_conv(j)
                if j >= 1:
                    emit_fc(j - 1)
            emit_fc(N_BLOCKS - 1)

    nc.finalize()
    return nc


def prepare_inputs(x, conv_w, W, b):
    SA, SB = build_conv_mats(conv_w)

    blob = np.zeros((128, BLOB_COLS), np.float32)
    blob[0:56, OFF_SA:OFF_SA + G2] = SA
    blob[64:120, OFF_SA:OFF_SA + G2] = SA
    blob[0:56, OFF_SB:OFF_SB + G2] = SB
    blob[64:120, OFF_SB:OFF_SB + G2] = SB

    Wf = np.asarray(W, np.float32)
    for t in range(N_FC):
        c0 = OFF_WP + FCM * t
        blob[0:G2, c0:c0 + NOUT] = Wf[G2 * 2 * t:G2 * (2 * t + 1), :]
        if t < N_FC - 1:
            blob[64:64 + G2, c0:c0 + NOUT] = Wf[G2 * (2 * t + 1):
                                                G2 * (2 * t + 2), :]
    blob[:, OFF_SEL:OFF_SEL + NOUT] = build_selector()
    blob = blob.astype(BF16)

    bias = np.asarray(b, np.float32).reshape(NOUT, 1)

    # Pack x: [B, 784] -> per core [120, N_BLOCKS, 7, NB] bf16
    # (partition-major across blocks for contiguous super-chunk DMAs).
    xbf = np.asarray(x, np.float32).astype(BF16)
    # [core, block, b, row, col] view of the batch-major input
    xv = xbf.reshape(N_CORES, N_BLOCKS, NB, IMG, IMG)
    in_maps = []
    for core in range(N_CORES):
        xp = np.zeros((XPART, N_BLOCKS, N_PASS, NB), BF16)
        for c in range(N_PASS):
            for r in range(2):
                # lo: chunk c rows 2c+r; hi: chunk c+7 rows 2c+14+r
                xp[r * IMG:(r + 1) * IMG, :, c, :] = (
                    xv[core, :, :, 2 * c + r, :].transpose(2, 0, 1)
                )
                xp[64 + r * IMG:64 + (r + 1) * IMG, :, c, :] = (
                    xv[core, :, :, 2 * c + 14 + r, :].transpose(2, 0, 1)
                )
        in_maps.append({"xP": xp, "blob": blob, "bias": bias})
    return in_maps


def run(x, conv_w, W, b, trace=False, **spmd_kwargs):
    in_maps = prepare_inputs(x, conv_w, W, b)
    nc = build_program()
    res = run_bass_kernel_spmd(
        nc, in_maps, list(range(N_CORES)), trace=trace, **spmd_kwargs
    )
    out = np.empty((B_FULL, NOUT), np.float32)
    for c in range(N_CORES):
        out[c * B_CORE:(c + 1) * B_CORE, :] = (
            res.results[c]["outT"].reshape(NOUT, B_CORE).T
        )
    return out, res


def kernel(x, conv_w, W, b):
    out, _ = run(x, conv_w, W, b, trace=False)
    return out


# revision 18
# speedup vs baseline: 1.4175x; 1.0474x over previous
"""Trainium2 Bass kernel for DigitConvolutionalModel.

Computes: out = relu(conv2d_valid(x.reshape(B,28,28), w3x3)).reshape(B,676) @ W + b

Strategy (pure data parallel over 8 NeuronCores, 8192 images/core), bf16:
  - Host: pack x per core partition-major [120, 16 blocks, 7, 512] bf16.
    Pass-column c holds 2-row chunk c (rows 2c,2c+1) at partitions 0..55
    and chunk c+7 (rows 2c+14,2c+15) at partitions 64..119.
  - x streams in 6 super-chunk DMAs (1,1,2,4,4,4 blocks) alternating the
    two HWDGE rings (sync/scalar); multi-block per-partition-contiguous
    descriptors (up to 28KB) amortize per-descriptor overhead. All of x
    stays resident in SBUF (~115KB/partition).
  - Conv per 512-image block: 7 passes. Pass p = 4 quadrant-tiled
    matmuls (2x2 tile_position grid, tile_size 64x64): chunk p lo rows x
    {A-taps -> group p, B-taps -> group p-1} and chunk p+7 hi rows x
    {A-taps -> group p+7, B-taps -> group p+6}. Groups g=0..12 are 2
    output rows (52 feats); group g accumulates A (chunk g) + B (chunk
    g+1) in PSUM partitions 64*(g%2)..+52 of pair bank g//2.
  - ReLU per pair bank [116,512] -> SBUF bf16 (gap partitions 52..63
    memset to 0 once per bank; FC weight rows there are 0).
  - FC: 7 matmuls (Wp chunks [116,32], col-tiled 4 strips) -> 2 rounds
    into one PSUM bank; DVE copy -> SBUF; Sel matmul [128,10] reduces
    the 4 col-strip partials; DVE bias-add; per-block outT store.
  - FC for block j emitted after conv of block j+1 (software pipelining).
"""

import numpy as np
import ml_dtypes

import concourse.bass as bass
import concourse.mybir as mybir
import concourse.tile as tile
from concourse import bacc
from concourse.bass_utils import run_bass_kernel_spmd

BF16 = ml_dtypes.bfloat16
PRECISION = "bf16"

# Problem geometry (fixed by the task spec)
B_FULL = 65536
IMG = 28
KW = 3
OH = IMG - KW + 1          # 26
NPIX = IMG * IMG           # 784
NFEAT = OH * OH            # 676
NOUT = 10

N_CORES = 8
B_CORE = B_FULL // N_CORES  # 8192
NB = 512                    # images per block
N_BLOCKS = B_CORE // NB     # 16

N_PASS = 7                  # conv passes per block (2 chunks each)
N_CHUNK = 14                # 2-row input chunks
N_GRP = 13                  # 2-out-row groups of 52 feats
G2 = 2 * OH                 # 52
XPART = 120                 # partitions used by the packed x layout
FCM = 32                    # FC stationary columns (one 32-col strip)
N_FC = 7                    # FC chunks (6 pairs of groups + 1 single)
KFC = 116                   # FC contraction for a group pair (52+12gap+52)

# Const blob column offsets (bf16, [128, BLOB_COLS])
OFF_SA = 0                  # [120, 52]  A-taps (lo rows 0..55, hi 64..119)
OFF_SB = 52                 # [120, 52]  B-taps
OFF_WP = 104                # [116, 7, 32] FC weights
OFF_SEL = 104 + 7 * FCM     # [128, 10]  col-strip reduction selector
BLOB_COLS = OFF_SEL + NOUT

WARM_MMS = 8                # HAM warm-up matmuls (open the clock gate
                            # during the first x super-chunk's transfer)

# x super-chunk sizes: small first (fast pipeline start). All x chunks go
# on the SWDGE (gpsimd) path in strict consumption order: per-SDMA-engine
# throughput on the HWDGE rings caps at ~15 GB/s, SWDGE measures higher,
# and a single ordered queue keeps arrival order aligned with consumption.
CHUNKS = (1, 1, 2, 2, 2, 2, 2, 2, 2)
LOOKAHEAD = 6               # issue chunk DMA when first block <= j + LOOKAHEAD


def build_conv_mats(conv_w: np.ndarray):
    """SA[pixel, feat]: taps of a group's own chunk (input rows 2g,2g+1).
    SB[pixel, feat]: taps of the next chunk (rows 2g+2,2g+3). feat =
    26*l + oj for out row 2g+l, col oj."""
    w = np.asarray(conv_w, np.float32)
    SA = np.zeros((56, G2), np.float32)
    SB = np.zeros((56, G2), np.float32)
    for l in range(2):
        for oj in range(OH):
            f = OH * l + oj
            for r in range(2):
                for dj in range(KW):
                    c = oj + dj
                    diA = r - l
                    if 0 <= diA < KW:
                        SA[r * IMG + c, f] = w[diA, dj]
                    diB = 2 + r - l
                    if 0 <= diB < KW:
                        SB[r * IMG + c, f] = w[diB, dj]
    return SA, SB


def build_selector() -> np.ndarray:
    """S[32j + o, o] = 1: sums the 4 col-strip FC partials."""
    S = np.zeros((128, NOUT), np.float32)
    for j in range(4):
        for o in range(NOUT):
            S[FCM * j + o, o] = 1.0
    return S


def build_program():
    f32 = mybir.dt.float32
    bf = mybir.dt.bfloat16

    nc = bacc.Bacc()
    # Partition-major across blocks: per-partition bytes for a k-block
    # super-chunk DMA are contiguous (k*7168B descriptors amortize the
    # ~190ns per-descriptor overhead that caps small-descriptor DMAs).
    xP = nc.declare_dram_parameter("xP", [XPART, N_BLOCKS, N_PASS, NB], bf,
                                   isOutput=False)
    blob_d = nc.declare_dram_parameter("blob", [128, BLOB_COLS], bf,
                                       isOutput=False)
    bias_d = nc.declare_dram_parameter("bias", [NOUT, 1], f32, isOutput=False)
    outT = nc.declare_dram_parameter("outT", [NOUT, N_BLOCKS, NB], f32,
                                     isOutput=True)

    Relu = mybir.ActivationFunctionType.Relu

    # pair index of the relu emitted after each conv pass
    RELU_AFTER = {2: [0], 3: [4], 4: [1], 5: [5], 6: [2, 3, 6]}
    RELU_ON_ACT = {0, 1, 2, 6}  # pairs relu'd on ACT; rest on DVE

    with tile.TileContext(nc) as tc:
        with (
            tc.tile_pool(name="const", bufs=1) as const,
            tc.tile_pool(name="feat", bufs=14) as fpool,
            tc.tile_pool(name="fcsb", bufs=2) as fcpool,
            tc.tile_pool(name="osb", bufs=3) as opool,
            tc.tile_pool(name="psc", bufs=7, space="PSUM") as psc,
            tc.tile_pool(name="pso", bufs=1, space="PSUM") as pso,
        ):
            # One DMA for all bf16 constants (~150KB, lands ~2.5us), one
            # tiny SWDGE DMA for the f32 bias.
            blob = const.tile([128, BLOB_COLS], bf)
            nc.sync.dma_start(out=blob[:, :], in_=blob_d[:, :])
            bias_sb = const.tile([NOUT, 1], f32)
            nc.sync.dma_start(out=bias_sb[:, :], in_=bias_d[:, :])

            def SA(rb):   # stationary A-taps at row base rb (0 or 64)
                return blob[rb:rb + 56, OFF_SA:OFF_SA + G2]

            def SB(rb):
                return blob[rb:rb + 56, OFF_SB:OFF_SB + G2]

            def WP(t, k):
                return blob[0:k, OFF_WP + FCM * t:OFF_WP + FCM * (t + 1)]

            sel = blob[:, OFF_SEL:OFF_SEL + NOUT]

            # PE warm-up during the first x super-chunk's transfer. Source
            # is DVE-memset (no DMA dependency) so warm-up starts right
            # after the engine preamble and opens the HAM clock gate.
            wsrc = const.tile([128, NB], bf)
            nc.vector.memset(wsrc[:, :], 0)
            warm_ps = psc.tile([128, NB], mybir.dt.float32, tag="convps")
            for _ in range(WARM_MMS):
                nc.tensor.matmul(
                    warm_ps[:, :], wsrc[:, :128], wsrc[:, :],
                    start=True, stop=True,
                )
            # Zero the psum gap partitions (52..63) of all 7 conv banks
            # once: ReLU reads [0:116] across the gap; FC weight rows
            # 52..63 are 0, but relu(NaN-garbage) would poison the FC.
            for _ in range(7):
                t = psc.tile([128, NB], mybir.dt.float32, tag="convps",
                             name="gapz")
                nc.vector.memset(t[:, :], 0)

            # x super-chunk DMAs are issued lazily inside the block loop
            # (program order near their consumers keeps the per-engine
            # instruction streams from serializing them behind compute).
            blk2tile = {}
            chunk_plan = []
            b0 = 0
            for ci, nb in enumerate(CHUNKS):
                chunk_plan.append((ci, b0, nb))
                b0 += nb
            next_chunk = [0]

            def issue_chunks(j):
                while next_chunk[0] < len(chunk_plan):
                    ci, b0, nb = chunk_plan[next_chunk[0]]
                    if b0 > j + LOOKAHEAD:
                        break
                    xt = const.tile([XPART, nb, N_PASS, NB], bf, name=f"xchunk{ci}")
                    eng = nc.sync if ci % 2 == 0 else nc.scalar
                    eng.dma_start(out=xt[:, :, :, :],
                                  in_=xP[:, b0:b0 + nb, :, :])
                    for b in range(nb):
                        blk2tile[b0 + b] = (xt, b)
                    next_chunk[0] += 1

            state = {}

            def emit_conv(j):
                xt, b = blk2tile[j]
                pairs = {}
                written = {}

                def pt(g):
                    k = g // 2
                    if k not in pairs:
                        pairs[k] = psc.tile([128, NB], mybir.dt.float32,
                                            tag="convps", name=f"pair{k}")
                    s = 64 * (g % 2)
                    return pairs[k][s:s + G2, :]

                def mm(g, stat, rb):
                    first = g not in written
                    written[g] = True
                    nc.tensor.matmul(
                        pt(g), stat(rb), xt[rb:rb + 56, b, p, :],
                        start=first, stop=not first,
                        tile_position=(rb, 64 * (g % 2)),
                        skip_group_check=True,
                    )

                feats = {}
                for p in range(N_PASS):
                    mm(p, SA, 0)                    # A: chunk p -> group p
                    if p >= 1:
                        mm(p - 1, SB, 0)            # B: chunk p -> group p-1
                    if p <= 5:
                        mm(p + 7, SA, 64)           # A: chunk p+7 -> group p+7
                    mm(p + 6, SB, 64)               # B: chunk p+7 -> group p+6
                    for k in RELU_AFTER.get(p, ()):
                        kf = KFC if k < N_FC - 1 else G2
                        ft = fpool.tile([kf, NB], bf, tag="feat", name=f"ft{k}")
                        if k in RELU_ON_ACT:
                            nc.scalar.activation(
                                ft[:, :], pairs[k][:kf, :], Relu)
                        else:
                            nc.vector.tensor_scalar_max(
                                ft[:, :], pairs[k][:kf, :], 0.0)
                        feats[k] = ft
                state[j] = feats

            def emit_fc(j):
                feats = state.pop(j)
                # 7 col-tiled matmuls into one PSUM bank: round 1 strips
                # 0..3 (each clears its strip), round 2 strips 0..2 accum.
                ops = pso.tile([128, NB], mybir.dt.float32, tag="outps")
                for t in range(N_FC):
                    kf = KFC if t < N_FC - 1 else G2
                    strip = FCM * (t % 4)
                    nc.tensor.matmul(
                        ops[strip:strip + FCM, :], WP(t, kf),
                        feats[t][:, :],
                        start=(t < 4), stop=(t >= 3),
                        tile_position=(0, strip), skip_group_check=True,
                    )
                fcsb = fcpool.tile([128, NB], bf, tag="fcsb")
                nc.vector.tensor_copy(fcsb[:, :], ops[:, :])
                # Sel output reuses the fc-partial bank (already copied out).
                nc.tensor.matmul(
                    ops[:NOUT, :], sel[:, :], fcsb[:, :], start=True,
                    stop=True, skip_group_check=True,
                )
                osb = opool.tile([NOUT, NB], f32, tag="osb")
                nc.vector.tensor_scalar(
                    osb[:, :], ops[:NOUT, :], bias_sb[:, :], None,
                    op0=mybir.AluOpType.add,
                )
                eng = nc.scalar if j % 2 == 0 else nc.sync
                eng.dma_start(out=outT[:, j, :], in_=osb[:, :])

            for j in range(N_BLOCKS):
                issue_chunks(j)
                emit_conv(j)
                if j >= 1:
                    emit_fc(j - 1)
            emit_fc(N_BLOCKS - 1)

    nc.finalize()
    return nc


def prepare_inputs(x, conv_w, W, b):
    SA, SB = build_conv_mats(conv_w)

    blob = np.zeros((128, BLOB_COLS), np.float32)
    blob[0:56, OFF_SA:OFF_SA + G2] = SA
    blob[64:120, OFF_SA:OFF_SA + G2] = SA
    blob[0:56, OFF_SB:OFF_SB + G2] = SB
    blob[64:120, OFF_SB:OFF_SB + G2] = SB

    Wf = np.asarray(W, np.float32)
    for t in range(N_FC):
        c0 = OFF_WP + FCM * t
        blob[0:G2, c0:c0 + NOUT] = Wf[G2 * 2 * t:G2 * (2 * t + 1), :]
        if t < N_FC - 1:
            blob[64:64 + G2, c0:c0 + NOUT] = Wf[G2 * (2 * t + 1):
                                                G2 * (2 * t + 2), :]
    blob[:, OFF_SEL:OFF_SEL + NOUT] = build_selector()
    blob = blob.astype(BF16)

    bias = np.asarray(b, np.float32).reshape(NOUT, 1)

    # Pack x: [B, 784] -> per core [120, N_BLOCKS, 7, NB] bf16
    # (partition-major across blocks for contiguous super-chunk DMAs).
    xbf = np.asarray(x, np.float32).astype(BF16)
    # [core, block, b, row, col] view of the batch-major input
    xv = xbf.reshape(N_CORES, N_BLOCKS, NB, IMG, IMG)
    in_maps = []
    for core in range(N_CORES):
        xp = np.zeros((XPART, N_BLOCKS, N_PASS, NB), BF16)
        for c in range(N_PASS):
            for r in range(2):
                # lo: chunk c rows 2c+r; hi: chunk c+7 rows 2c+14+r
                xp[r * IMG:(r + 1) * IMG, :, c, :] = (
                    xv[core, :, :, 2 * c + r, :].transpose(2, 0, 1)
                )
                xp[64 + r * IMG:64 + (r + 1) * IMG, :, c, :] = (
                    xv[core, :, :, 2 * c + 14 + r, :].transpose(2, 0, 1)
                )
        in_maps.append({"xP": xp, "blob": blob, "bias": bias})
    return in_maps


def run(x, conv_w, W, b, trace=False, **spmd_kwargs):
    in_maps = prepare_inputs(x, conv_w, W, b)
    nc = build_program()
    res = run_bass_kernel_spmd(
        nc, in_maps, list(range(N_CORES)), trace=trace, **spmd_kwargs
    )
    out = np.empty((B_FULL, NOUT), np.float32)
    for c in range(N_CORES):
        out[c * B_CORE:(c + 1) * B_CORE, :] = (
            res.results[c]["outT"].reshape(NOUT, B_CORE).T
        )
    return out, res


def kernel(x, conv_w, W, b):
    out, _ = run(x, conv_w, W, b, trace=False)
    return out


# revision 19
# speedup vs baseline: 1.7655x; 1.2455x over previous
"""Trainium2 Bass kernel for DigitConvolutionalModel.

Computes: out = relu(conv2d_valid(x.reshape(B,28,28), w3x3)).reshape(B,676) @ W + b

Strategy (pure data parallel over 8 NeuronCores, 8192 images/core), bf16:
  - Host: pack x per core partition-major [120, 16 blocks, 7, 512] bf16.
    Pass-column c holds 2-row chunk c (rows 2c,2c+1) at partitions 0..55
    and chunk c+7 (rows 2c+14,2c+15) at partitions 64..119.
  - x streams in 6 super-chunk DMAs (1,1,2,4,4,4 blocks) alternating the
    two HWDGE rings (sync/scalar); multi-block per-partition-contiguous
    descriptors (up to 28KB) amortize per-descriptor overhead. All of x
    stays resident in SBUF (~115KB/partition).
  - Conv per 512-image block: 7 passes. Pass p = 4 quadrant-tiled
    matmuls (2x2 tile_position grid, tile_size 64x64): chunk p lo rows x
    {A-taps -> group p, B-taps -> group p-1} and chunk p+7 hi rows x
    {A-taps -> group p+7, B-taps -> group p+6}. Groups g=0..12 are 2
    output rows (52 feats); group g accumulates A (chunk g) + B (chunk
    g+1) in PSUM partitions 64*(g%2)..+52 of pair bank g//2.
  - ReLU per pair bank [116,512] -> SBUF bf16 (gap partitions 52..63
    memset to 0 once per bank; FC weight rows there are 0).
  - FC: 7 matmuls (Wp chunks [116,32], col-tiled 4 strips) -> 2 rounds
    into one PSUM bank; DVE copy -> SBUF; Sel matmul [128,10] reduces
    the 4 col-strip partials; DVE bias-add; per-block outT store.
  - FC for block j emitted after conv of block j+1 (software pipelining).
"""

import numpy as np
import ml_dtypes

import concourse.bass as bass
import concourse.mybir as mybir
import concourse.tile as tile
from concourse import bacc
from concourse.bass_utils import run_bass_kernel_spmd

BF16 = ml_dtypes.bfloat16
PRECISION = "bf16"

# Problem geometry (fixed by the task spec)
B_FULL = 65536
IMG = 28
KW = 3
OH = IMG - KW + 1          # 26
NPIX = IMG * IMG           # 784
NFEAT = OH * OH            # 676
NOUT = 10

N_CORES = 8
B_CORE = B_FULL // N_CORES  # 8192
NB = 512                    # images per block
N_BLOCKS = B_CORE // NB     # 16

N_PASS = 7                  # conv passes per block (2 chunks each)
N_CHUNK = 14                # 2-row input chunks
N_GRP = 13                  # 2-out-row groups of 52 feats
G2 = 2 * OH                 # 52
XPART = 120                 # partitions used by the packed x layout
FCM = 32                    # FC stationary columns (one 32-col strip)
N_FC = 7                    # FC chunks (6 pairs of groups + 1 single)
KFC = 116                   # FC contraction for a group pair (52+12gap+52)

# Const blob column offsets (bf16, [128, BLOB_COLS])
OFF_SA = 0                  # [120, 52]  A-taps (lo rows 0..55, hi 64..119)
OFF_SB = 52                 # [120, 52]  B-taps
OFF_WP = 104                # [116, 7, 32] FC weights
OFF_SEL = 104 + 7 * FCM     # [128, 10]  col-strip reduction selector
BLOB_COLS = OFF_SEL + NOUT

WARM_MMS = 8                # HAM warm-up matmuls (open the clock gate
                            # during the first x super-chunk's transfer)

# Per-block x DMAs, split at the partition gap: lo partitions [0:56] ride
# the sync HWDGE ring (even SDMA engines), hi partitions [64:120] ride the
# gpsimd SWDGE queue (odd engines). Skips the zero gap partitions 56..63
# (-6.7% bytes). dma_start for block j+1 is emitted AFTER emit_conv(j) so
# each conv's DMA-wait threshold covers only blocks <= j (the scheduler
# bundles all earlier-emitted DMAs on a lane into the wait threshold).


def build_conv_mats(conv_w: np.ndarray):
    """SA[pixel, feat]: taps of a group's own chunk (input rows 2g,2g+1).
    SB[pixel, feat]: taps of the next chunk (rows 2g+2,2g+3). feat =
    26*l + oj for out row 2g+l, col oj."""
    w = np.asarray(conv_w, np.float32)
    SA = np.zeros((56, G2), np.float32)
    SB = np.zeros((56, G2), np.float32)
    for l in range(2):
        for oj in range(OH):
            f = OH * l + oj
            for r in range(2):
                for dj in range(KW):
                    c = oj + dj
                    diA = r - l
                    if 0 <= diA < KW:
                        SA[r * IMG + c, f] = w[diA, dj]
                    diB = 2 + r - l
                    if 0 <= diB < KW:
                        SB[r * IMG + c, f] = w[diB, dj]
    return SA, SB


def build_selector() -> np.ndarray:
    """S[32j + o, o] = 1: sums the 4 col-strip FC partials."""
    S = np.zeros((128, NOUT), np.float32)
    for j in range(4):
        for o in range(NOUT):
            S[FCM * j + o, o] = 1.0
    return S


def build_program():
    f32 = mybir.dt.float32
    bf = mybir.dt.bfloat16

    nc = bacc.Bacc()
    # Partition-major across blocks: per-partition bytes for a k-block
    # super-chunk DMA are contiguous (k*7168B descriptors amortize the
    # ~190ns per-descriptor overhead that caps small-descriptor DMAs).
    xP = nc.declare_dram_parameter("xP", [XPART, N_BLOCKS, N_PASS, NB], bf,
                                   isOutput=False)
    blob_d = nc.declare_dram_parameter("blob", [128, BLOB_COLS], bf,
                                       isOutput=False)
    bias_d = nc.declare_dram_parameter("bias", [NOUT, 1], f32, isOutput=False)
    outT = nc.declare_dram_parameter("outT", [NOUT, N_BLOCKS, NB], f32,
                                     isOutput=True)

    Relu = mybir.ActivationFunctionType.Relu

    # pair index of the relu emitted after each conv pass
    RELU_AFTER = {2: [0], 3: [4], 4: [1], 5: [5], 6: [2, 3, 6]}
    RELU_ON_ACT = {0, 1, 2, 6}  # pairs relu'd on ACT; rest on DVE

    with tile.TileContext(nc) as tc:
        with (
            tc.tile_pool(name="const", bufs=1) as const,
            tc.tile_pool(name="feat", bufs=14) as fpool,
            tc.tile_pool(name="fcsb", bufs=2) as fcpool,
            tc.tile_pool(name="osb", bufs=3) as opool,
            tc.tile_pool(name="psc", bufs=7, space="PSUM") as psc,
            tc.tile_pool(name="pso", bufs=1, space="PSUM") as pso,
        ):
            # Constants on the scalar ring (sync ring is reserved for the
            # x-lo stream): one blob DMA (~150KB) plus the tiny f32 bias.
            blob = const.tile([128, BLOB_COLS], bf)
            nc.scalar.dma_start(out=blob[:, :], in_=blob_d[:, :])
            bias_sb = const.tile([NOUT, 1], f32)
            nc.scalar.dma_start(out=bias_sb[:, :], in_=bias_d[:, :])

            def SA(rb):   # stationary A-taps at row base rb (0 or 64)
                return blob[rb:rb + 56, OFF_SA:OFF_SA + G2]

            def SB(rb):
                return blob[rb:rb + 56, OFF_SB:OFF_SB + G2]

            def WP(t, k):
                return blob[0:k, OFF_WP + FCM * t:OFF_WP + FCM * (t + 1)]

            sel = blob[:, OFF_SEL:OFF_SEL + NOUT]

            # PE warm-up during the first x super-chunk's transfer. Source
            # is DVE-memset (no DMA dependency) so warm-up starts right
            # after the engine preamble and opens the HAM clock gate.
            wsrc = const.tile([128, NB], bf)
            nc.vector.memset(wsrc[:, :], 0)
            warm_ps = psc.tile([128, NB], mybir.dt.float32, tag="convps")
            for _ in range(WARM_MMS):
                nc.tensor.matmul(
                    warm_ps[:, :], wsrc[:, :128], wsrc[:, :],
                    start=True, stop=True,
                )
            # Zero the psum gap partitions (52..63) of all 7 conv banks
            # once: ReLU reads [0:116] across the gap; FC weight rows
            # 52..63 are 0, but relu(NaN-garbage) would poison the FC.
            for _ in range(7):
                t = psc.tile([128, NB], mybir.dt.float32, tag="convps",
                             name="gapz")
                nc.vector.memset(t[:, :], 0)

            xts = {}

            def issue_x(j):
                if j >= N_BLOCKS or j in xts:
                    return
                xt = const.tile([XPART, N_PASS, NB], bf, name=f"xb{j}")
                nc.sync.dma_start(out=xt[0:56, :, :],
                                  in_=xP[0:56, j, :, :])
                nc.gpsimd.dma_start(out=xt[64:120, :, :],
                                    in_=xP[64:120, j, :, :])
                xts[j] = xt

            state = {}

            def emit_conv(j):
                xt = xts[j]
                pairs = {}
                written = {}

                def pt(g):
                    k = g // 2
                    if k not in pairs:
                        pairs[k] = psc.tile([128, NB], mybir.dt.float32,
                                            tag="convps", name=f"pair{k}")
                    s = 64 * (g % 2)
                    return pairs[k][s:s + G2, :]

                def mm(g, stat, rb):
                    first = g not in written
                    written[g] = True
                    nc.tensor.matmul(
                        pt(g), stat(rb), xt[rb:rb + 56, p, :],
                        start=first, stop=not first,
                        tile_position=(rb, 64 * (g % 2)),
                        skip_group_check=True,
                    )

                feats = {}
                for p in range(N_PASS):
                    mm(p, SA, 0)                    # A: chunk p -> group p
                    if p >= 1:
                        mm(p - 1, SB, 0)            # B: chunk p -> group p-1
                    if p <= 5:
                        mm(p + 7, SA, 64)           # A: chunk p+7 -> group p+7
                    mm(p + 6, SB, 64)               # B: chunk p+7 -> group p+6
                    for k in RELU_AFTER.get(p, ()):
                        kf = KFC if k < N_FC - 1 else G2
                        ft = fpool.tile([kf, NB], bf, tag="feat", name=f"ft{k}")
                        if k in RELU_ON_ACT:
                            nc.scalar.activation(
                                ft[:, :], pairs[k][:kf, :], Relu)
                        else:
                            nc.vector.tensor_scalar_max(
                                ft[:, :], pairs[k][:kf, :], 0.0)
                        feats[k] = ft
                state[j] = feats

            def emit_fc(j):
                feats = state.pop(j)
                # 7 col-tiled matmuls into one PSUM bank: round 1 strips
                # 0..3 (each clears its strip), round 2 strips 0..2 accum.
                ops = pso.tile([128, NB], mybir.dt.float32, tag="outps")
                for t in range(N_FC):
                    kf = KFC if t < N_FC - 1 else G2
                    strip = FCM * (t % 4)
                    nc.tensor.matmul(
                        ops[strip:strip + FCM, :], WP(t, kf),
                        feats[t][:, :],
                        start=(t < 4), stop=(t >= 3),
                        tile_position=(0, strip), skip_group_check=True,
                    )
                fcsb = fcpool.tile([128, NB], bf, tag="fcsb")
                nc.vector.tensor_copy(fcsb[:, :], ops[:, :])
                # Sel output reuses the fc-partial bank (already copied out).
                nc.tensor.matmul(
                    ops[:NOUT, :], sel[:, :], fcsb[:, :], start=True,
                    stop=True, skip_group_check=True,
                )
                osb = opool.tile([NOUT, NB], f32, tag="osb")
                nc.vector.tensor_scalar(
                    osb[:, :], ops[:NOUT, :], bias_sb[:, :], None,
                    op0=mybir.AluOpType.add,
                )
                nc.scalar.dma_start(out=outT[:, j, :], in_=osb[:, :])

            issue_x(0)
            for j in range(N_BLOCKS):
                emit_conv(j)
                issue_x(j + 1)
                if j >= 1:
                    emit_fc(j - 1)
            emit_fc(N_BLOCKS - 1)

    nc.finalize()
    return nc


def prepare_inputs(x, conv_w, W, b):
    SA, SB = build_conv_mats(conv_w)

    blob = np.zeros((128, BLOB_COLS), np.float32)
    blob[0:56, OFF_SA:OFF_SA + G2] = SA
    blob[64:120, OFF_SA:OFF_SA + G2] = SA
    blob[0:56, OFF_SB:OFF_SB + G2] = SB
    blob[64:120, OFF_SB:OFF_SB + G2] = SB

    Wf = np.asarray(W, np.float32)
    for t in range(N_FC):
        c0 = OFF_WP + FCM * t
        blob[0:G2, c0:c0 + NOUT] = Wf[G2 * 2 * t:G2 * (2 * t + 1), :]
        if t < N_FC - 1:
            blob[64:64 + G2, c0:c0 + NOUT] = Wf[G2 * (2 * t + 1):
                                                G2 * (2 * t + 2), :]
    blob[:, OFF_SEL:OFF_SEL + NOUT] = build_selector()
    blob = blob.astype(BF16)

    bias = np.asarray(b, np.float32).reshape(NOUT, 1)

    # Pack x: [B, 784] -> per core [120, N_BLOCKS, 7, NB] bf16
    # (partition-major across blocks for contiguous super-chunk DMAs).
    xbf = np.asarray(x, np.float32).astype(BF16)
    # [core, block, b, row, col] view of the batch-major input
    xv = xbf.reshape(N_CORES, N_BLOCKS, NB, IMG, IMG)
    in_maps = []
    for core in range(N_CORES):
        xp = np.zeros((XPART, N_BLOCKS, N_PASS, NB), BF16)
        for c in range(N_PASS):
            for r in range(2):
                # lo: chunk c rows 2c+r; hi: chunk c+7 rows 2c+14+r
                xp[r * IMG:(r + 1) * IMG, :, c, :] = (
                    xv[core, :, :, 2 * c + r, :].transpose(2, 0, 1)
                )
                xp[64 + r * IMG:64 + (r + 1) * IMG, :, c, :] = (
                    xv[core, :, :, 2 * c + 14 + r, :].transpose(2, 0, 1)
                )
        in_maps.append({"xP": xp, "blob": blob, "bias": bias})
    return in_maps


def run(x, conv_w, W, b, trace=False, **spmd_kwargs):
    in_maps = prepare_inputs(x, conv_w, W, b)
    nc = build_program()
    res = run_bass_kernel_spmd(
        nc, in_maps, list(range(N_CORES)), trace=trace, **spmd_kwargs
    )
    out = np.empty((B_FULL, NOUT), np.float32)
    for c in range(N_CORES):
        out[c * B_CORE:(c + 1) * B_CORE, :] = (
            res.results[c]["outT"].reshape(NOUT, B_CORE).T
        )
    return out, res


def kernel(x, conv_w, W, b):
    out, _ = run(x, conv_w, W, b, trace=False)
    return out


# revision 20
# speedup vs baseline: 1.9282x; 1.0921x over previous
"""Trainium2 Bass kernel for DigitConvolutionalModel.

Computes: out = relu(conv2d_valid(x.reshape(B,28,28), w3x3)).reshape(B,676) @ W + b

Strategy (pure data parallel over 8 NeuronCores, 8192 images/core), bf16:
  - Host: pack x per core partition-major [120, 16 blocks, 7, 512] bf16.
    Pass-column c holds 2-row chunk c (rows 2c,2c+1) at partitions 0..55
    and chunk c+7 (rows 2c+14,2c+15) at partitions 64..119.
  - x streams in 6 super-chunk DMAs (1,1,2,4,4,4 blocks) alternating the
    two HWDGE rings (sync/scalar); multi-block per-partition-contiguous
    descriptors (up to 28KB) amortize per-descriptor overhead. All of x
    stays resident in SBUF (~115KB/partition).
  - Conv per 512-image block: 7 passes. Pass p = 4 quadrant-tiled
    matmuls (2x2 tile_position grid, tile_size 64x64): chunk p lo rows x
    {A-taps -> group p, B-taps -> group p-1} and chunk p+7 hi rows x
    {A-taps -> group p+7, B-taps -> group p+6}. Groups g=0..12 are 2
    output rows (52 feats); group g accumulates A (chunk g) + B (chunk
    g+1) in PSUM partitions 64*(g%2)..+52 of pair bank g//2.
  - ReLU per pair bank [116,512] -> SBUF bf16 (gap partitions 52..63
    memset to 0 once per bank; FC weight rows there are 0).
  - FC: 7 matmuls (Wp chunks [116,32], col-tiled 4 strips) -> 2 rounds
    into one PSUM bank; DVE copy -> SBUF; Sel matmul [128,10] reduces
    the 4 col-strip partials; DVE bias-add; per-block outT store.
  - FC for block j emitted after conv of block j+1 (software pipelining).
"""

import numpy as np
import ml_dtypes

import concourse.bass as bass
import concourse.mybir as mybir
import concourse.tile as tile
from concourse import bacc
from concourse.bass_utils import run_bass_kernel_spmd

BF16 = ml_dtypes.bfloat16
PRECISION = "bf16"

# Problem geometry (fixed by the task spec)
B_FULL = 65536
IMG = 28
KW = 3
OH = IMG - KW + 1          # 26
NPIX = IMG * IMG           # 784
NFEAT = OH * OH            # 676
NOUT = 10

N_CORES = 8
B_CORE = B_FULL // N_CORES  # 8192
NB = 512                    # images per block
N_BLOCKS = B_CORE // NB     # 16

N_PASS = 7                  # conv passes per block (2 chunks each)
N_CHUNK = 14                # 2-row input chunks
N_GRP = 13                  # 2-out-row groups of 52 feats
G2 = 2 * OH                 # 52
XPART = 120                 # partitions used by the packed x layout
FCM = 32                    # FC stationary columns (one 32-col strip)
N_FC = 7                    # FC chunks (6 pairs of groups + 1 single)
KFC = 116                   # FC contraction for a group pair (52+12gap+52)

# Const blob column offsets (bf16, [128, BLOB_COLS])
OFF_SA = 0                  # [120, 52]  A-taps (lo rows 0..55, hi 64..119)
OFF_SB = 52                 # [120, 52]  B-taps
OFF_WP = 104                # [116, 7, 32] FC weights
OFF_SEL = 104 + 7 * FCM     # [128, 10]  col-strip reduction selector
BLOB_COLS = OFF_SEL + NOUT

WARM_MMS = 8                # HAM warm-up matmuls (open the clock gate
                            # during the first x super-chunk's transfer)

# Per-block x DMAs, split at the partition gap: lo partitions [0:56] ride
# the sync HWDGE ring (even SDMA engines), hi partitions [64:120] ride the
# gpsimd SWDGE queue (odd engines). Skips the zero gap partitions 56..63
# (-6.7% bytes). dma_start for block j+1 is emitted AFTER emit_conv(j) so
# each conv's DMA-wait threshold covers only blocks <= j (the scheduler
# bundles all earlier-emitted DMAs on a lane into the wait threshold).


def build_conv_mats(conv_w: np.ndarray):
    """SA[pixel, feat]: taps of a group's own chunk (input rows 2g,2g+1).
    SB[pixel, feat]: taps of the next chunk (rows 2g+2,2g+3). feat =
    26*l + oj for out row 2g+l, col oj."""
    w = np.asarray(conv_w, np.float32)
    SA = np.zeros((56, G2), np.float32)
    SB = np.zeros((56, G2), np.float32)
    for l in range(2):
        for oj in range(OH):
            f = OH * l + oj
            for r in range(2):
                for dj in range(KW):
                    c = oj + dj
                    diA = r - l
                    if 0 <= diA < KW:
                        SA[r * IMG + c, f] = w[diA, dj]
                    diB = 2 + r - l
                    if 0 <= diB < KW:
                        SB[r * IMG + c, f] = w[diB, dj]
    return SA, SB


def build_selector() -> np.ndarray:
    """S[32j + o, o] = 1: sums the 4 col-strip FC partials."""
    S = np.zeros((128, NOUT), np.float32)
    for j in range(4):
        for o in range(NOUT):
            S[FCM * j + o, o] = 1.0
    return S


def build_program():
    f32 = mybir.dt.float32
    bf = mybir.dt.bfloat16

    nc = bacc.Bacc()
    # Partition-major across blocks: per-partition bytes for a k-block
    # super-chunk DMA are contiguous (k*7168B descriptors amortize the
    # ~190ns per-descriptor overhead that caps small-descriptor DMAs).
    xP = nc.declare_dram_parameter("xP", [XPART, N_BLOCKS, N_PASS, NB], bf,
                                   isOutput=False)
    blob_d = nc.declare_dram_parameter("blob", [128, BLOB_COLS], bf,
                                       isOutput=False)
    bias_d = nc.declare_dram_parameter("bias", [NOUT, 1], f32, isOutput=False)
    outT = nc.declare_dram_parameter("outT", [NOUT, N_BLOCKS, NB], f32,
                                     isOutput=True)

    Relu = mybir.ActivationFunctionType.Relu

    # pair index of the relu emitted after each conv pass
    RELU_AFTER = {2: [0], 3: [4], 4: [1], 5: [5], 6: [2, 3, 6]}
    RELU_ON_ACT = {0, 1, 2, 6}  # pairs relu'd on ACT; rest on DVE

    with tile.TileContext(nc) as tc:
        with (
            tc.tile_pool(name="const", bufs=1) as const,
            tc.tile_pool(name="feat", bufs=14) as fpool,
            tc.tile_pool(name="fcsb", bufs=2) as fcpool,
            tc.tile_pool(name="osb", bufs=16) as opool,
            tc.tile_pool(name="psc", bufs=7, space="PSUM") as psc,
            tc.tile_pool(name="pso", bufs=1, space="PSUM") as pso,
        ):
            # Constants on the scalar ring (sync ring is reserved for the
            # x-lo stream): one blob DMA (~150KB) plus the tiny f32 bias.
            blob = const.tile([128, BLOB_COLS], bf)
            nc.scalar.dma_start(out=blob[:, :], in_=blob_d[:, :])
            bias_sb = const.tile([NOUT, 1], f32)
            nc.scalar.dma_start(out=bias_sb[:, :], in_=bias_d[:, :])

            def SA(rb):   # stationary A-taps at row base rb (0 or 64)
                return blob[rb:rb + 56, OFF_SA:OFF_SA + G2]

            def SB(rb):
                return blob[rb:rb + 56, OFF_SB:OFF_SB + G2]

            def WP(t, k):
                return blob[0:k, OFF_WP + FCM * t:OFF_WP + FCM * (t + 1)]

            sel = blob[:, OFF_SEL:OFF_SEL + NOUT]

            # PE warm-up during the first x super-chunk's transfer. Source
            # is DVE-memset (no DMA dependency) so warm-up starts right
            # after the engine preamble and opens the HAM clock gate.
            wsrc = const.tile([128, NB], bf)
            nc.vector.memset(wsrc[:, :], 0)
            warm_ps = psc.tile([128, NB], mybir.dt.float32, tag="convps")
            for _ in range(WARM_MMS):
                nc.tensor.matmul(
                    warm_ps[:, :], wsrc[:, :128], wsrc[:, :],
                    start=True, stop=True,
                )
            # Zero the psum gap partitions (52..63) of all 7 conv banks
            # once: ReLU reads [0:116] across the gap; FC weight rows
            # 52..63 are 0, but relu(NaN-garbage) would poison the FC.
            for _ in range(7):
                t = psc.tile([128, NB], mybir.dt.float32, tag="convps",
                             name="gapz")
                nc.vector.memset(t[:, :], 0)

            xts = {}

            def issue_x(j):
                if j >= N_BLOCKS or j in xts:
                    return
                xt = const.tile([XPART, N_PASS, NB], bf, name=f"xb{j}")
                nc.sync.dma_start(out=xt[0:56, :, :],
                                  in_=xP[0:56, j, :, :])
                nc.gpsimd.dma_start(out=xt[64:120, :, :],
                                    in_=xP[64:120, j, :, :])
                xts[j] = xt

            state = {}

            def emit_conv(j):
                xt = xts[j]
                pairs = {}
                written = {}

                def pt(g):
                    k = g // 2
                    if k not in pairs:
                        pairs[k] = psc.tile([128, NB], mybir.dt.float32,
                                            tag="convps", name=f"pair{k}")
                    s = 64 * (g % 2)
                    return pairs[k][s:s + G2, :]

                def mm(g, stat, rb):
                    first = g not in written
                    written[g] = True
                    nc.tensor.matmul(
                        pt(g), stat(rb), xt[rb:rb + 56, p, :],
                        start=first, stop=not first,
                        tile_position=(rb, 64 * (g % 2)),
                        skip_group_check=True,
                    )

                feats = {}
                for p in range(N_PASS):
                    mm(p, SA, 0)                    # A: chunk p -> group p
                    if p >= 1:
                        mm(p - 1, SB, 0)            # B: chunk p -> group p-1
                    if p <= 5:
                        mm(p + 7, SA, 64)           # A: chunk p+7 -> group p+7
                    mm(p + 6, SB, 64)               # B: chunk p+7 -> group p+6
                    for k in RELU_AFTER.get(p, ()):
                        kf = KFC if k < N_FC - 1 else G2
                        ft = fpool.tile([kf, NB], bf, tag="feat", name=f"ft{k}")
                        if k in RELU_ON_ACT:
                            nc.scalar.activation(
                                ft[:, :], pairs[k][:kf, :], Relu)
                        else:
                            nc.vector.tensor_scalar_max(
                                ft[:, :], pairs[k][:kf, :], 0.0)
                        feats[k] = ft
                state[j] = feats

            def emit_fc(j):
                feats = state.pop(j)
                # 7 col-tiled matmuls into one PSUM bank: round 1 strips
                # 0..3 (each clears its strip), round 2 strips 0..2 accum.
                ops = pso.tile([128, NB], mybir.dt.float32, tag="outps")
                for t in range(N_FC):
                    kf = KFC if t < N_FC - 1 else G2
                    strip = FCM * (t % 4)
                    nc.tensor.matmul(
                        ops[strip:strip + FCM, :], WP(t, kf),
                        feats[t][:, :],
                        start=(t < 4), stop=(t >= 3),
                        tile_position=(0, strip), skip_group_check=True,
                    )
                fcsb = fcpool.tile([128, NB], bf, tag="fcsb")
                nc.vector.tensor_copy(fcsb[:, :], ops[:, :])
                # Sel output reuses the fc-partial bank (already copied out).
                nc.tensor.matmul(
                    ops[:NOUT, :], sel[:, :], fcsb[:, :], start=True,
                    stop=True, skip_group_check=True,
                )
                osb = opool.tile([NOUT, NB], f32, tag="osb")
                nc.vector.tensor_scalar(
                    osb[:, :], ops[:NOUT, :], bias_sb[:, :], None,
                    op0=mybir.AluOpType.add,
                )
                osbs[j] = osb

            osbs = {}
            issue_x(0)
            for j in range(N_BLOCKS):
                emit_conv(j)
                issue_x(j + 1)
                if j >= 1:
                    emit_fc(j - 1)
            emit_fc(N_BLOCKS - 1)
            # Output stores last, on the sync sequencer (its x descgens all
            # run early): each store's descriptor-gen waits only its own
            # bias result, so stores pace with compute and never head-of-
            # line-block the relu stream or pollute x DMA-lane thresholds.
            for j in range(N_BLOCKS):
                nc.sync.dma_start(out=outT[:, j, :], in_=osbs[j][:, :])

    nc.finalize()
    return nc


def prepare_inputs(x, conv_w, W, b):
    SA, SB = build_conv_mats(conv_w)

    blob = np.zeros((128, BLOB_COLS), np.float32)
    blob[0:56, OFF_SA:OFF_SA + G2] = SA
    blob[64:120, OFF_SA:OFF_SA + G2] = SA
    blob[0:56, OFF_SB:OFF_SB + G2] = SB
    blob[64:120, OFF_SB:OFF_SB + G2] = SB

    Wf = np.asarray(W, np.float32)
    for t in range(N_FC):
        c0 = OFF_WP + FCM * t
        blob[0:G2, c0:c0 + NOUT] = Wf[G2 * 2 * t:G2 * (2 * t + 1), :]
        if t < N_FC - 1:
            blob[64:64 + G2, c0:c0 + NOUT] = Wf[G2 * (2 * t + 1):
                                                G2 * (2 * t + 2), :]
    blob[:, OFF_SEL:OFF_SEL + NOUT] = build_selector()
    blob = blob.astype(BF16)

    bias = np.asarray(b, np.float32).reshape(NOUT, 1)

    # Pack x: [B, 784] -> per core [120, N_BLOCKS, 7, NB] bf16
    # (partition-major across blocks for contiguous super-chunk DMAs).
    xbf = np.asarray(x, np.float32).astype(BF16)
    # [core, block, b, row, col] view of the batch-major input
    xv = xbf.reshape(N_CORES, N_BLOCKS, NB, IMG, IMG)
    in_maps = []
    for core in range(N_CORES):
        xp = np.zeros((XPART, N_BLOCKS, N_PASS, NB), BF16)
        for c in range(N_PASS):
            for r in range(2):
                # lo: chunk c rows 2c+r; hi: chunk c+7 rows 2c+14+r
                xp[r * IMG:(r + 1) * IMG, :, c, :] = (
                    xv[core, :, :, 2 * c + r, :].transpose(2, 0, 1)
                )
                xp[64 + r * IMG:64 + (r + 1) * IMG, :, c, :] = (
                    xv[core, :, :, 2 * c + 14 + r, :].transpose(2, 0, 1)
                )
        in_maps.append({"xP": xp, "blob": blob, "bias": bias})
    return in_maps


def run(x, conv_w, W, b, trace=False, **spmd_kwargs):
    in_maps = prepare_inputs(x, conv_w, W, b)
    nc = build_program()
    res = run_bass_kernel_spmd(
        nc, in_maps, list(range(N_CORES)), trace=trace, **spmd_kwargs
    )
    out = np.empty((B_FULL, NOUT), np.float32)
    for c in range(N_CORES):
        out[c * B_CORE:(c + 1) * B_CORE, :] = (
            res.results[c]["outT"].reshape(NOUT, B_CORE).T
        )
    return out, res


def kernel(x, conv_w, W, b):
    out, _ = run(x, conv_w, W, b, trace=False)
    return out


# revision 21
# speedup vs baseline: 1.9924x; 1.0333x over previous
"""Trainium2 Bass kernel for DigitConvolutionalModel.

Computes: out = relu(conv2d_valid(x.reshape(B,28,28), w3x3)).reshape(B,676) @ W + b

Strategy (pure data parallel over 8 NeuronCores, 8192 images/core), bf16:
  - Host: pack x per core partition-major [120, 16 blocks, 7, 512] bf16.
    Pass-column c holds 2-row chunk c (rows 2c,2c+1) at partitions 0..55
    and chunk c+7 (rows 2c+14,2c+15) at partitions 64..119.
  - x streams in 6 super-chunk DMAs (1,1,2,4,4,4 blocks) alternating the
    two HWDGE rings (sync/scalar); multi-block per-partition-contiguous
    descriptors (up to 28KB) amortize per-descriptor overhead. All of x
    stays resident in SBUF (~115KB/partition).
  - Conv per 512-image block: 7 passes. Pass p = 4 quadrant-tiled
    matmuls (2x2 tile_position grid, tile_size 64x64): chunk p lo rows x
    {A-taps -> group p, B-taps -> group p-1} and chunk p+7 hi rows x
    {A-taps -> group p+7, B-taps -> group p+6}. Groups g=0..12 are 2
    output rows (52 feats); group g accumulates A (chunk g) + B (chunk
    g+1) in PSUM partitions 64*(g%2)..+52 of pair bank g//2.
  - ReLU per pair bank [116,512] -> SBUF bf16 (gap partitions 52..63
    memset to 0 once per bank; FC weight rows there are 0).
  - FC: 7 matmuls (Wp chunks [116,32], col-tiled 4 strips) -> 2 rounds
    into one PSUM bank; DVE copy -> SBUF; Sel matmul [128,10] reduces
    the 4 col-strip partials; DVE bias-add; per-block outT store.
  - FC for block j emitted after conv of block j+1 (software pipelining).
"""

import numpy as np
import ml_dtypes

import concourse.bass as bass
import concourse.mybir as mybir
import concourse.tile as tile
from concourse import bacc
from concourse.bass_utils import run_bass_kernel_spmd

BF16 = ml_dtypes.bfloat16
PRECISION = "bf16"

# Problem geometry (fixed by the task spec)
B_FULL = 65536
IMG = 28
KW = 3
OH = IMG - KW + 1          # 26
NPIX = IMG * IMG           # 784
NFEAT = OH * OH            # 676
NOUT = 10

N_CORES = 8
B_CORE = B_FULL // N_CORES  # 8192
NB = 512                    # images per block
N_BLOCKS = B_CORE // NB     # 16

N_PASS = 7                  # conv passes per block (2 chunks each)
N_CHUNK = 14                # 2-row input chunks
N_GRP = 13                  # 2-out-row groups of 52 feats
G2 = 2 * OH                 # 52
XPART = 120                 # partitions used by the packed x layout
FCM = 32                    # FC stationary columns (one 32-col strip)
N_FC = 7                    # FC chunks (6 pairs of groups + 1 single)
KFC = 116                   # FC contraction for a group pair (52+12gap+52)

# Const blob column offsets (bf16, [128, BLOB_COLS])
OFF_SA = 0                  # [120, 52]  A-taps (lo rows 0..55, hi 64..119)
OFF_SB = 52                 # [120, 52]  B-taps
OFF_WP = 104                # [116, 7, 32] FC weights
OFF_SEL = 104 + 7 * FCM     # [128, 10]  col-strip reduction selector
BLOB_COLS = OFF_SEL + NOUT

WARM_MMS = 13               # HAM warm-up matmuls: open the clock gate AND
                            # bridge until x-hi(0) lands (~13.5us) so the PE
                            # never idles long enough to re-throttle

# Per-block x DMAs, split at the partition gap: lo partitions [0:56] ride
# the sync HWDGE ring (even SDMA engines), hi partitions [64:120] ride the
# gpsimd SWDGE queue (odd engines). Skips the zero gap partitions 56..63
# (-6.7% bytes). dma_start for block j+1 is emitted AFTER emit_conv(j) so
# each conv's DMA-wait threshold covers only blocks <= j (the scheduler
# bundles all earlier-emitted DMAs on a lane into the wait threshold).


def build_conv_mats(conv_w: np.ndarray):
    """SA[pixel, feat]: taps of a group's own chunk (input rows 2g,2g+1).
    SB[pixel, feat]: taps of the next chunk (rows 2g+2,2g+3). feat =
    26*l + oj for out row 2g+l, col oj."""
    w = np.asarray(conv_w, np.float32)
    SA = np.zeros((56, G2), np.float32)
    SB = np.zeros((56, G2), np.float32)
    for l in range(2):
        for oj in range(OH):
            f = OH * l + oj
            for r in range(2):
                for dj in range(KW):
                    c = oj + dj
                    diA = r - l
                    if 0 <= diA < KW:
                        SA[r * IMG + c, f] = w[diA, dj]
                    diB = 2 + r - l
                    if 0 <= diB < KW:
                        SB[r * IMG + c, f] = w[diB, dj]
    return SA, SB


def build_selector() -> np.ndarray:
    """S[32j + o, o] = 1: sums the 4 col-strip FC partials."""
    S = np.zeros((128, NOUT), np.float32)
    for j in range(4):
        for o in range(NOUT):
            S[FCM * j + o, o] = 1.0
    return S


def build_program():
    f32 = mybir.dt.float32
    bf = mybir.dt.bfloat16

    nc = bacc.Bacc()
    # Partition-major across blocks: per-partition bytes for a k-block
    # super-chunk DMA are contiguous (k*7168B descriptors amortize the
    # ~190ns per-descriptor overhead that caps small-descriptor DMAs).
    xP = nc.declare_dram_parameter("xP", [XPART, N_BLOCKS, N_PASS, NB], bf,
                                   isOutput=False)
    blob_d = nc.declare_dram_parameter("blob", [128, BLOB_COLS], bf,
                                       isOutput=False)
    bias_d = nc.declare_dram_parameter("bias", [NOUT, 1], f32, isOutput=False)
    outT = nc.declare_dram_parameter("outT", [NOUT, N_BLOCKS, NB], f32,
                                     isOutput=True)

    Relu = mybir.ActivationFunctionType.Relu

    # pair index of the relu emitted after each conv pass
    RELU_AFTER = {2: [0], 3: [4], 4: [1], 5: [5], 6: [2, 3, 6]}
    RELU_ON_ACT = {0, 1, 2, 6}  # pairs relu'd on ACT; rest on DVE

    with tile.TileContext(nc) as tc:
        with (
            tc.tile_pool(name="const", bufs=1) as const,
            tc.tile_pool(name="feat", bufs=14) as fpool,
            tc.tile_pool(name="fcsb", bufs=2) as fcpool,
            tc.tile_pool(name="osb", bufs=16) as opool,
            tc.tile_pool(name="psc", bufs=6, space="PSUM") as psc,
            tc.tile_pool(name="pso", bufs=2, space="PSUM") as pso,
        ):
            # Constants on the scalar ring (sync ring is reserved for the
            # x-lo stream): one blob DMA (~150KB) plus the tiny f32 bias.
            blob = const.tile([128, BLOB_COLS], bf)
            nc.scalar.dma_start(out=blob[:, :], in_=blob_d[:, :])
            bias_sb = const.tile([NOUT, 1], f32)
            nc.scalar.dma_start(out=bias_sb[:, :], in_=bias_d[:, :])

            def SA(rb):   # stationary A-taps at row base rb (0 or 64)
                return blob[rb:rb + 56, OFF_SA:OFF_SA + G2]

            def SB(rb):
                return blob[rb:rb + 56, OFF_SB:OFF_SB + G2]

            def WP(t, k):
                return blob[0:k, OFF_WP + FCM * t:OFF_WP + FCM * (t + 1)]

            sel = blob[:, OFF_SEL:OFF_SEL + NOUT]

            # PE warm-up during the first x super-chunk's transfer. Source
            # is DVE-memset (no DMA dependency) so warm-up starts right
            # after the engine preamble and opens the HAM clock gate.
            wsrc = const.tile([128, NB], bf)
            nc.vector.memset(wsrc[:, :], 0)
            warm_ps = psc.tile([128, NB], mybir.dt.float32, tag="convps")
            for _ in range(WARM_MMS):
                nc.tensor.matmul(
                    warm_ps[:, :], wsrc[:, :128], wsrc[:, :],
                    start=True, stop=True,
                )
            # Zero the psum gap partitions (52..63) of all 7 conv banks
            # once: ReLU reads [0:116] across the gap; FC weight rows
            # 52..63 are 0, but relu(NaN-garbage) would poison the FC.
            for _ in range(7):
                t = psc.tile([128, NB], mybir.dt.float32, tag="convps",
                             name="gapz")
                nc.vector.memset(t[:, :], 0)

            xts = {}

            def issue_x(j):
                if j >= N_BLOCKS or j in xts:
                    return
                xt = const.tile([XPART, N_PASS, NB], bf, name=f"xb{j}")
                nc.sync.dma_start(out=xt[0:56, :, :],
                                  in_=xP[0:56, j, :, :])
                nc.gpsimd.dma_start(out=xt[64:120, :, :],
                                    in_=xP[64:120, j, :, :])
                xts[j] = xt

            state = {}

            def emit_conv(j):
                xt = xts[j]
                pairs = {}
                written = {}

                def pt(g):
                    k = g // 2
                    if k not in pairs:
                        pairs[k] = psc.tile([128, NB], mybir.dt.float32,
                                            tag="convps", name=f"pair{k}")
                    s = 64 * (g % 2)
                    return pairs[k][s:s + G2, :]

                def mm(g, stat, rb):
                    first = g not in written
                    written[g] = True
                    nc.tensor.matmul(
                        pt(g), stat(rb), xt[rb:rb + 56, p, :],
                        start=first, stop=not first,
                        tile_position=(rb, 64 * (g % 2)),
                        skip_group_check=True,
                    )

                feats = {}
                for p in range(N_PASS):
                    mm(p, SA, 0)                    # A: chunk p -> group p
                    if p >= 1:
                        mm(p - 1, SB, 0)            # B: chunk p -> group p-1
                    if p <= 5:
                        mm(p + 7, SA, 64)           # A: chunk p+7 -> group p+7
                    mm(p + 6, SB, 64)               # B: chunk p+7 -> group p+6
                    for k in RELU_AFTER.get(p, ()):
                        kf = KFC if k < N_FC - 1 else G2
                        ft = fpool.tile([kf, NB], bf, tag="feat", name=f"ft{k}")
                        if k in RELU_ON_ACT:
                            nc.scalar.activation(
                                ft[:, :], pairs[k][:kf, :], Relu)
                        else:
                            nc.vector.tensor_scalar_max(
                                ft[:, :], pairs[k][:kf, :], 0.0)
                        feats[k] = ft
                state[j] = feats

            def emit_fc(j):
                feats = state.pop(j)
                # 7 col-tiled matmuls into one PSUM bank: round 1 strips
                # 0..3 (each clears its strip), round 2 strips 0..2 accum.
                ops = pso.tile([128, NB], mybir.dt.float32, tag="outps")
                for t in range(N_FC):
                    kf = KFC if t < N_FC - 1 else G2
                    strip = FCM * (t % 4)
                    nc.tensor.matmul(
                        ops[strip:strip + FCM, :], WP(t, kf),
                        feats[t][:, :],
                        start=(t < 4), stop=(t >= 3),
                        tile_position=(0, strip), skip_group_check=True,
                    )
                fcsb = fcpool.tile([128, NB], bf, tag="fcsb")
                nc.vector.tensor_copy(fcsb[:, :], ops[:, :])
                # Sel output reuses the fc-partial bank (already copied out).
                nc.tensor.matmul(
                    ops[:NOUT, :], sel[:, :], fcsb[:, :], start=True,
                    stop=True, skip_group_check=True,
                )
                osb = opool.tile([NOUT, NB], f32, tag="osb")
                nc.vector.tensor_scalar(
                    osb[:, :], ops[:NOUT, :], bias_sb[:, :], None,
                    op0=mybir.AluOpType.add,
                )
                osbs[j] = osb

            osbs = {}
            issue_x(0)
            for j in range(N_BLOCKS):
                emit_conv(j)
                issue_x(j + 1)
                if j >= 1:
                    emit_fc(j - 1)
            emit_fc(N_BLOCKS - 1)
            # Output stores last, on the sync sequencer (its x descgens all
            # run early): each store's descriptor-gen waits only its own
            # bias result, so stores pace with compute and never head-of-
            # line-block the relu stream or pollute x DMA-lane thresholds.
            for j in range(N_BLOCKS):
                nc.sync.dma_start(out=outT[:, j, :], in_=osbs[j][:, :])

    nc.finalize()
    return nc


def prepare_inputs(x, conv_w, W, b):
    SA, SB = build_conv_mats(conv_w)

    blob = np.zeros((128, BLOB_COLS), np.float32)
    blob[0:56, OFF_SA:OFF_SA + G2] = SA
    blob[64:120, OFF_SA:OFF_SA + G2] = SA
    blob[0:56, OFF_SB:OFF_SB + G2] = SB
    blob[64:120, OFF_SB:OFF_SB + G2] = SB

    Wf = np.asarray(W, np.float32)
    for t in range(N_FC):
        c0 = OFF_WP + FCM * t
        blob[0:G2, c0:c0 + NOUT] = Wf[G2 * 2 * t:G2 * (2 * t + 1), :]
        if t < N_FC - 1:
            blob[64:64 + G2, c0:c0 + NOUT] = Wf[G2 * (2 * t + 1):
                                                G2 * (2 * t + 2), :]
    blob[:, OFF_SEL:OFF_SEL + NOUT] = build_selector()
    blob = blob.astype(BF16)

    bias = np.asarray(b, np.float32).reshape(NOUT, 1)

    # Pack x: [B, 784] -> per core [120, N_BLOCKS, 7, NB] bf16
    # (partition-major across blocks for contiguous super-chunk DMAs).
    xbf = np.asarray(x, np.float32).astype(BF16)
    # [core, block, b, row, col] view of the batch-major input
    xv = xbf.reshape(N_CORES, N_BLOCKS, NB, IMG, IMG)
    in_maps = []
    for core in range(N_CORES):
        xp = np.zeros((XPART, N_BLOCKS, N_PASS, NB), BF16)
        for c in range(N_PASS):
            for r in range(2):
                # lo: chunk c rows 2c+r; hi: chunk c+7 rows 2c+14+r
                xp[r * IMG:(r + 1) * IMG, :, c, :] = (
                    xv[core, :, :, 2 * c + r, :].transpose(2, 0, 1)
                )
                xp[64 + r * IMG:64 + (r + 1) * IMG, :, c, :] = (
                    xv[core, :, :, 2 * c + 14 + r, :].transpose(2, 0, 1)
                )
        in_maps.append({"xP": xp, "blob": blob, "bias": bias})
    return in_maps


def run(x, conv_w, W, b, trace=False, **spmd_kwargs):
    in_maps = prepare_inputs(x, conv_w, W, b)
    nc = build_program()
    res = run_bass_kernel_spmd(
        nc, in_maps, list(range(N_CORES)), trace=trace, **spmd_kwargs
    )
    out = np.empty((B_FULL, NOUT), np.float32)
    for c in range(N_CORES):
        out[c * B_CORE:(c + 1) * B_CORE, :] = (
            res.results[c]["outT"].reshape(NOUT, B_CORE).T
        )
    return out, res


def kernel(x, conv_w, W, b):
    out, _ = run(x, conv_w, W, b, trace=False)
    return out


# revision 22
# speedup vs baseline: 2.0906x; 1.0493x over previous
"""Trainium2 Bass kernel for DigitConvolutionalModel.

Computes: out = relu(conv2d_valid(x.reshape(B,28,28), w3x3)).reshape(B,676) @ W + b

Strategy (pure data parallel over 8 NeuronCores, 8192 images/core), bf16:
  - Host: pack x per core partition-major [120, 16 blocks, 7, 512] bf16.
    Pass-column c holds 2-row chunk c (rows 2c,2c+1) at partitions 0..55
    and chunk c+7 (rows 2c+14,2c+15) at partitions 64..119.
  - x streams in 6 super-chunk DMAs (1,1,2,4,4,4 blocks) alternating the
    two HWDGE rings (sync/scalar); multi-block per-partition-contiguous
    descriptors (up to 28KB) amortize per-descriptor overhead. All of x
    stays resident in SBUF (~115KB/partition).
  - Conv per 512-image block: 7 passes. Pass p = 4 quadrant-tiled
    matmuls (2x2 tile_position grid, tile_size 64x64): chunk p lo rows x
    {A-taps -> group p, B-taps -> group p-1} and chunk p+7 hi rows x
    {A-taps -> group p+7, B-taps -> group p+6}. Groups g=0..12 are 2
    output rows (52 feats); group g accumulates A (chunk g) + B (chunk
    g+1) in PSUM partitions 64*(g%2)..+52 of pair bank g//2.
  - ReLU per pair bank [116,512] -> SBUF bf16 (gap partitions 52..63
    memset to 0 once per bank; FC weight rows there are 0).
  - FC: 7 matmuls (Wp chunks [116,32], col-tiled 4 strips) -> 2 rounds
    into one PSUM bank; DVE copy -> SBUF; Sel matmul [128,10] reduces
    the 4 col-strip partials; DVE bias-add; per-block outT store.
  - FC for block j emitted after conv of block j+1 (software pipelining).
"""

import numpy as np
import ml_dtypes

import concourse.bass as bass
import concourse.mybir as mybir
import concourse.tile as tile
from concourse import bacc
from concourse.bass_utils import run_bass_kernel_spmd

BF16 = ml_dtypes.bfloat16
PRECISION = "bf16"

# Problem geometry (fixed by the task spec)
B_FULL = 65536
IMG = 28
KW = 3
OH = IMG - KW + 1          # 26
NPIX = IMG * IMG           # 784
NFEAT = OH * OH            # 676
NOUT = 10

N_CORES = 8
B_CORE = B_FULL // N_CORES  # 8192
NB = 512                    # images per block
N_BLOCKS = B_CORE // NB     # 16

N_PASS = 7                  # conv passes per block (2 chunks each)
N_CHUNK = 14                # 2-row input chunks
N_GRP = 13                  # 2-out-row groups of 52 feats
G2 = 2 * OH                 # 52
XPART = 120                 # partitions used by the packed x layout
FCM = 32                    # FC stationary columns (one 32-col strip)
N_FC = 7                    # FC chunks (6 pairs of groups + 1 single)
KFC = 116                   # FC contraction for a group pair (52+12gap+52)

# Const blob column offsets (bf16, [128, BLOB_COLS])
OFF_SA = 0                  # [120, 52]  A-taps (lo rows 0..55, hi 64..119)
OFF_SB = 52                 # [120, 52]  B-taps
OFF_WP = 104                # [116, 7, 32] FC weights
OFF_SEL = 104 + 7 * FCM     # [128, 10]  col-strip reduction selector
BLOB_COLS = OFF_SEL + NOUT

WARM_MMS = 13               # HAM warm-up matmuls: open the clock gate AND
                            # bridge until x-hi(0) lands (~13.5us) so the PE
                            # never idles long enough to re-throttle

# Per-block x DMAs, split at the partition gap: lo partitions [0:56] ride
# the sync HWDGE ring (even SDMA engines), hi partitions [64:120] ride the
# gpsimd SWDGE queue (odd engines). Skips the zero gap partitions 56..63
# (-6.7% bytes). dma_start for block j+1 is emitted AFTER emit_conv(j) so
# each conv's DMA-wait threshold covers only blocks <= j (the scheduler
# bundles all earlier-emitted DMAs on a lane into the wait threshold).


def build_conv_mats(conv_w: np.ndarray):
    """SA[pixel, feat]: taps of a group's own chunk (input rows 2g,2g+1).
    SB[pixel, feat]: taps of the next chunk (rows 2g+2,2g+3). feat =
    26*l + oj for out row 2g+l, col oj."""
    w = np.asarray(conv_w, np.float32)
    SA = np.zeros((56, G2), np.float32)
    SB = np.zeros((56, G2), np.float32)
    for l in range(2):
        for oj in range(OH):
            f = OH * l + oj
            for r in range(2):
                for dj in range(KW):
                    c = oj + dj
                    diA = r - l
                    if 0 <= diA < KW:
                        SA[r * IMG + c, f] = w[diA, dj]
                    diB = 2 + r - l
                    if 0 <= diB < KW:
                        SB[r * IMG + c, f] = w[diB, dj]
    return SA, SB


def build_selector() -> np.ndarray:
    """S[32j + o, o] = 1: sums the 4 col-strip FC partials."""
    S = np.zeros((128, NOUT), np.float32)
    for j in range(4):
        for o in range(NOUT):
            S[FCM * j + o, o] = 1.0
    return S


def build_program():
    f32 = mybir.dt.float32
    bf = mybir.dt.bfloat16

    nc = bacc.Bacc()
    # Partition-major across blocks: per-partition bytes for a k-block
    # super-chunk DMA are contiguous (k*7168B descriptors amortize the
    # ~190ns per-descriptor overhead that caps small-descriptor DMAs).
    xP = nc.declare_dram_parameter("xP", [XPART, N_BLOCKS, N_PASS, NB], bf,
                                   isOutput=False)
    blob_d = nc.declare_dram_parameter("blob", [128, BLOB_COLS], bf,
                                       isOutput=False)
    bias_d = nc.declare_dram_parameter("bias", [NOUT, 1], f32, isOutput=False)
    outT = nc.declare_dram_parameter("outT", [NOUT, N_BLOCKS, NB], f32,
                                     isOutput=True)

    Relu = mybir.ActivationFunctionType.Relu

    # pair index of the relu emitted after each conv pass
    RELU_AFTER = {2: [0], 3: [4], 4: [1], 5: [5], 6: [2, 3, 6]}
    RELU_ON_ACT = {0, 1, 2, 3, 6}  # pairs relu'd on ACT; rest on DVE

    with tile.TileContext(nc) as tc:
        with (
            tc.tile_pool(name="const", bufs=1) as const,
            tc.tile_pool(name="feat", bufs=14) as fpool,
            tc.tile_pool(name="fcsb", bufs=2) as fcpool,
            tc.tile_pool(name="osb", bufs=16) as opool,
            tc.tile_pool(name="psc", bufs=6, space="PSUM") as psc,
            tc.tile_pool(name="pso", bufs=2, space="PSUM") as pso,
        ):
            # Constants on the scalar ring (sync ring is reserved for the
            # x-lo stream): one blob DMA (~150KB) plus the tiny f32 bias.
            blob = const.tile([128, BLOB_COLS], bf)
            nc.scalar.dma_start(out=blob[:, :], in_=blob_d[:, :])
            bias_sb = const.tile([NOUT, 1], f32)
            nc.scalar.dma_start(out=bias_sb[:, :], in_=bias_d[:, :])

            def SA(rb):   # stationary A-taps at row base rb (0 or 64)
                return blob[rb:rb + 56, OFF_SA:OFF_SA + G2]

            def SB(rb):
                return blob[rb:rb + 56, OFF_SB:OFF_SB + G2]

            def WP(t, k):
                return blob[0:k, OFF_WP + FCM * t:OFF_WP + FCM * (t + 1)]

            sel = blob[:, OFF_SEL:OFF_SEL + NOUT]

            # PE warm-up during the first x super-chunk's transfer. Source
            # is DVE-memset (no DMA dependency) so warm-up starts right
            # after the engine preamble and opens the HAM clock gate.
            wsrc = const.tile([128, NB], bf)
            nc.vector.memset(wsrc[:, :], 0)
            warm_ps = psc.tile([128, NB], mybir.dt.float32, tag="convps")
            for _ in range(WARM_MMS):
                nc.tensor.matmul(
                    warm_ps[:, :], wsrc[:, :128], wsrc[:, :],
                    start=True, stop=True,
                )
            # Zero the psum gap partitions (52..63) of all 7 conv banks
            # once: ReLU reads [0:116] across the gap; FC weight rows
            # 52..63 are 0, but relu(NaN-garbage) would poison the FC.
            for _ in range(7):
                t = psc.tile([128, NB], mybir.dt.float32, tag="convps",
                             name="gapz")
                nc.vector.memset(t[:, :], 0)

            xts = {}

            def issue_x(j):
                if j >= N_BLOCKS or j in xts:
                    return
                xt = const.tile([XPART, N_PASS, NB], bf, name=f"xb{j}")
                nc.sync.dma_start(out=xt[0:56, :, :],
                                  in_=xP[0:56, j, :, :])
                nc.gpsimd.dma_start(out=xt[64:120, :, :],
                                    in_=xP[64:120, j, :, :])
                xts[j] = xt

            state = {}

            def emit_conv(j):
                xt = xts[j]
                pairs = {}
                written = {}

                def pt(g):
                    k = g // 2
                    if k not in pairs:
                        pairs[k] = psc.tile([128, NB], mybir.dt.float32,
                                            tag="convps", name=f"pair{k}")
                    s = 64 * (g % 2)
                    return pairs[k][s:s + G2, :]

                def mm(g, stat, rb):
                    first = g not in written
                    written[g] = True
                    nc.tensor.matmul(
                        pt(g), stat(rb), xt[rb:rb + 56, p, :],
                        start=first, stop=not first,
                        tile_position=(rb, 64 * (g % 2)),
                        skip_group_check=True,
                    )

                feats = {}
                for p in range(N_PASS):
                    mm(p, SA, 0)                    # A: chunk p -> group p
                    if p >= 1:
                        mm(p - 1, SB, 0)            # B: chunk p -> group p-1
                    if p <= 5:
                        mm(p + 7, SA, 64)           # A: chunk p+7 -> group p+7
                    mm(p + 6, SB, 64)               # B: chunk p+7 -> group p+6
                    for k in RELU_AFTER.get(p, ()):
                        kf = KFC if k < N_FC - 1 else G2
                        ft = fpool.tile([kf, NB], bf, tag="feat", name=f"ft{k}")
                        if k in RELU_ON_ACT:
                            nc.scalar.activation(
                                ft[:, :], pairs[k][:kf, :], Relu)
                        else:
                            nc.vector.tensor_scalar_max(
                                ft[:, :], pairs[k][:kf, :], 0.0)
                        feats[k] = ft
                state[j] = feats

            def emit_fc(j):
                feats = state.pop(j)
                # 7 col-tiled matmuls into one PSUM bank: round 1 strips
                # 0..3 (each clears its strip), round 2 strips 0..2 accum.
                ops = pso.tile([128, NB], mybir.dt.float32, tag="outps")
                for t in range(N_FC):
                    kf = KFC if t < N_FC - 1 else G2
                    strip = FCM * (t % 4)
                    nc.tensor.matmul(
                        ops[strip:strip + FCM, :], WP(t, kf),
                        feats[t][:, :],
                        start=(t < 4), stop=(t >= 3),
                        tile_position=(0, strip), skip_group_check=True,
                    )
                fcsb = fcpool.tile([128, NB], bf, tag="fcsb")
                nc.vector.tensor_copy(fcsb[:, :], ops[:, :])
                # Sel output reuses the fc-partial bank (already copied out).
                nc.tensor.matmul(
                    ops[:NOUT, :], sel[:, :], fcsb[:, :], start=True,
                    stop=True, skip_group_check=True,
                )
                osb = opool.tile([NOUT, NB], f32, tag="osb")
                nc.vector.tensor_scalar(
                    osb[:, :], ops[:NOUT, :], bias_sb[:, :], None,
                    op0=mybir.AluOpType.add,
                )
                osbs[j] = osb

            osbs = {}
            issue_x(0)
            for j in range(N_BLOCKS):
                emit_conv(j)
                issue_x(j + 1)
                if j >= 1:
                    emit_fc(j - 1)
            emit_fc(N_BLOCKS - 1)
            # Output stores last, on the sync sequencer (its x descgens all
            # run early): each store's descriptor-gen waits only its own
            # bias result, so stores pace with compute and never head-of-
            # line-block the relu stream or pollute x DMA-lane thresholds.
            for j in range(N_BLOCKS):
                nc.sync.dma_start(out=outT[:, j, :], in_=osbs[j][:, :])

    nc.finalize()
    return nc


def prepare_inputs(x, conv_w, W, b):
    SA, SB = build_conv_mats(conv_w)

    blob = np.zeros((128, BLOB_COLS), np.float32)
    blob[0:56, OFF_SA:OFF_SA + G2] = SA
    blob[64:120, OFF_SA:OFF_SA + G2] = SA
    blob[0:56, OFF_SB:OFF_SB + G2] = SB
    blob[64:120, OFF_SB:OFF_SB + G2] = SB

    Wf = np.asarray(W, np.float32)
    for t in range(N_FC):
        c0 = OFF_WP + FCM * t
        blob[0:G2, c0:c0 + NOUT] = Wf[G2 * 2 * t:G2 * (2 * t + 1), :]
        if t < N_FC - 1:
            blob[64:64 + G2, c0:c0 + NOUT] = Wf[G2 * (2 * t + 1):
                                                G2 * (2 * t + 2), :]
    blob[:, OFF_SEL:OFF_SEL + NOUT] = build_selector()
    blob = blob.astype(BF16)

    bias = np.asarray(b, np.float32).reshape(NOUT, 1)

    # Pack x: [B, 784] -> per core [120, N_BLOCKS, 7, NB] bf16
    # (partition-major across blocks for contiguous super-chunk DMAs).
    xbf = np.asarray(x, np.float32).astype(BF16)
    # [core, block, b, row, col] view of the batch-major input
    xv = xbf.reshape(N_CORES, N_BLOCKS, NB, IMG, IMG)
    in_maps = []
    for core in range(N_CORES):
        xp = np.zeros((XPART, N_BLOCKS, N_PASS, NB), BF16)
        for c in range(N_PASS):
            for r in range(2):
                # lo: chunk c rows 2c+r; hi: chunk c+7 rows 2c+14+r
                xp[r * IMG:(r + 1) * IMG, :, c, :] = (
                    xv[core, :, :, 2 * c + r, :].transpose(2, 0, 1)
                )
                xp[64 + r * IMG:64 + (r + 1) * IMG, :, c, :] = (
                    xv[core, :, :, 2 * c + 14 + r, :].transpose(2, 0, 1)
                )
        in_maps.append({"xP": xp, "blob": blob, "bias": bias})
    return in_maps


def run(x, conv_w, W, b, trace=False, **spmd_kwargs):
    in_maps = prepare_inputs(x, conv_w, W, b)
    nc = build_program()
    res = run_bass_kernel_spmd(
        nc, in_maps, list(range(N_CORES)), trace=trace, **spmd_kwargs
    )
    out = np.empty((B_FULL, NOUT), np.float32)
    for c in range(N_CORES):
        out[c * B_CORE:(c + 1) * B_CORE, :] = (
            res.results[c]["outT"].reshape(NOUT, B_CORE).T
        )
    return out, res


def kernel(x, conv_w, W, b):
    out, _ = run(x, conv_w, W, b, trace=False)
    return out


# revision 23
# speedup vs baseline: 2.1374x; 1.0224x over previous
"""Trainium2 Bass kernel for DigitConvolutionalModel.

Computes: out = relu(conv2d_valid(x.reshape(B,28,28), w3x3)).reshape(B,676) @ W + b

Strategy (pure data parallel over 8 NeuronCores, 8192 images/core), bf16:
  - Host: pack x per core partition-major [120, 16 blocks, 7, 512] bf16.
    Pass-column c holds 2-row chunk c (rows 2c,2c+1) at partitions 0..55
    and chunk c+7 (rows 2c+14,2c+15) at partitions 64..119.
  - x streams in 6 super-chunk DMAs (1,1,2,4,4,4 blocks) alternating the
    two HWDGE rings (sync/scalar); multi-block per-partition-contiguous
    descriptors (up to 28KB) amortize per-descriptor overhead. All of x
    stays resident in SBUF (~115KB/partition).
  - Conv per 512-image block: 7 passes. Pass p = 4 quadrant-tiled
    matmuls (2x2 tile_position grid, tile_size 64x64): chunk p lo rows x
    {A-taps -> group p, B-taps -> group p-1} and chunk p+7 hi rows x
    {A-taps -> group p+7, B-taps -> group p+6}. Groups g=0..12 are 2
    output rows (52 feats); group g accumulates A (chunk g) + B (chunk
    g+1) in PSUM partitions 64*(g%2)..+52 of pair bank g//2.
  - ReLU per pair bank [116,512] -> SBUF bf16 (gap partitions 52..63
    memset to 0 once per bank; FC weight rows there are 0).
  - FC: 7 matmuls (Wp chunks [116,32], col-tiled 4 strips) -> 2 rounds
    into one PSUM bank; DVE copy -> SBUF; Sel matmul [128,10] reduces
    the 4 col-strip partials; DVE bias-add; per-block outT store.
  - FC for block j emitted after conv of block j+1 (software pipelining).
"""

import numpy as np
import ml_dtypes

import concourse.bass as bass
import concourse.mybir as mybir
import concourse.tile as tile
from concourse import bacc
from concourse.bass_utils import run_bass_kernel_spmd

BF16 = ml_dtypes.bfloat16
PRECISION = "bf16"

# Problem geometry (fixed by the task spec)
B_FULL = 65536
IMG = 28
KW = 3
OH = IMG - KW + 1          # 26
NPIX = IMG * IMG           # 784
NFEAT = OH * OH            # 676
NOUT = 10

N_CORES = 8
B_CORE = B_FULL // N_CORES  # 8192
NB = 512                    # images per block
N_BLOCKS = B_CORE // NB     # 16

N_PASS = 7                  # conv passes per block (2 chunks each)
N_CHUNK = 14                # 2-row input chunks
N_GRP = 13                  # 2-out-row groups of 52 feats
G2 = 2 * OH                 # 52
XPART = 120                 # partitions used by the packed x layout
FCM = 32                    # FC stationary columns (one 32-col strip)
N_FC = 7                    # FC chunks (6 pairs of groups + 1 single)
KFC = 116                   # FC contraction for a group pair (52+12gap+52)

# Const blob column offsets (bf16, [128, BLOB_COLS])
OFF_SA = 0                  # [120, 52]  A-taps (lo rows 0..55, hi 64..119)
OFF_SB = 52                 # [120, 52]  B-taps
OFF_WP = 104                # [116, 7, 32] FC weights
OFF_SEL = 104 + 7 * FCM     # [128, 10]  col-strip reduction selector
BLOB_COLS = OFF_SEL + NOUT

WARM_MMS = 13               # HAM warm-up matmuls: open the clock gate AND
                            # bridge until x-hi(0) lands (~13.5us) so the PE
                            # never idles long enough to re-throttle

# Per-block x DMAs, split at the partition gap: lo partitions [0:56] ride
# the sync HWDGE ring (even SDMA engines), hi partitions [64:120] ride the
# gpsimd SWDGE queue (odd engines). Skips the zero gap partitions 56..63
# (-6.7% bytes). dma_start for block j+1 is emitted AFTER emit_conv(j) so
# each conv's DMA-wait threshold covers only blocks <= j (the scheduler
# bundles all earlier-emitted DMAs on a lane into the wait threshold).


def build_conv_mats(conv_w: np.ndarray):
    """SA[pixel, feat]: taps of a group's own chunk (input rows 2g,2g+1).
    SB[pixel, feat]: taps of the next chunk (rows 2g+2,2g+3). feat =
    26*l + oj for out row 2g+l, col oj."""
    w = np.asarray(conv_w, np.float32)
    SA = np.zeros((56, G2), np.float32)
    SB = np.zeros((56, G2), np.float32)
    for l in range(2):
        for oj in range(OH):
            f = OH * l + oj
            for r in range(2):
                for dj in range(KW):
                    c = oj + dj
                    diA = r - l
                    if 0 <= diA < KW:
                        SA[r * IMG + c, f] = w[diA, dj]
                    diB = 2 + r - l
                    if 0 <= diB < KW:
                        SB[r * IMG + c, f] = w[diB, dj]
    return SA, SB


def build_selector() -> np.ndarray:
    """S[32j + o, o] = 1: sums the 4 col-strip FC partials."""
    S = np.zeros((128, NOUT), np.float32)
    for j in range(4):
        for o in range(NOUT):
            S[FCM * j + o, o] = 1.0
    return S


def build_program():
    f32 = mybir.dt.float32
    bf = mybir.dt.bfloat16

    nc = bacc.Bacc()
    # Partition-major across blocks: per-partition bytes for a k-block
    # super-chunk DMA are contiguous (k*7168B descriptors amortize the
    # ~190ns per-descriptor overhead that caps small-descriptor DMAs).
    xP = nc.declare_dram_parameter("xP", [XPART, N_BLOCKS, N_PASS, NB], bf,
                                   isOutput=False)
    blob_d = nc.declare_dram_parameter("blob", [128, BLOB_COLS], bf,
                                       isOutput=False)
    bias_d = nc.declare_dram_parameter("bias", [NOUT, 1], f32, isOutput=False)
    outT = nc.declare_dram_parameter("outT", [NOUT, N_BLOCKS, NB], f32,
                                     isOutput=True)

    Relu = mybir.ActivationFunctionType.Relu

    # pair index of the relu emitted after each conv pass
    RELU_AFTER = {2: [0], 3: [4], 4: [1], 5: [5], 6: [2, 3, 6]}
    RELU_ON_ACT = {0, 1, 2, 3, 6}  # pairs relu'd on ACT; rest on DVE

    with tile.TileContext(nc) as tc:
        with (
            tc.tile_pool(name="const", bufs=1) as const,
            tc.tile_pool(name="feat", bufs=14) as fpool,
            tc.tile_pool(name="fcsb", bufs=2) as fcpool,
            tc.tile_pool(name="osb", bufs=16) as opool,
            tc.tile_pool(name="psc", bufs=6, space="PSUM") as psc,
            tc.tile_pool(name="pso", bufs=2, space="PSUM") as pso,
        ):
            # Constants on the scalar ring (sync ring is reserved for the
            # x-lo stream): one blob DMA (~150KB) plus the tiny f32 bias.
            blob = const.tile([128, BLOB_COLS], bf)
            nc.scalar.dma_start(out=blob[:, :], in_=blob_d[:, :])
            bias_sb = const.tile([NOUT, 1], f32)
            nc.scalar.dma_start(out=bias_sb[:, :], in_=bias_d[:, :])

            def SA(rb):   # stationary A-taps at row base rb (0 or 64)
                return blob[rb:rb + 56, OFF_SA:OFF_SA + G2]

            def SB(rb):
                return blob[rb:rb + 56, OFF_SB:OFF_SB + G2]

            def WP(t, k):
                return blob[0:k, OFF_WP + FCM * t:OFF_WP + FCM * (t + 1)]

            sel = blob[:, OFF_SEL:OFF_SEL + NOUT]

            # PE warm-up during the first x super-chunk's transfer. Source
            # is DVE-memset (no DMA dependency) so warm-up starts right
            # after the engine preamble and opens the HAM clock gate.
            wsrc = const.tile([128, NB], bf)
            nc.vector.memset(wsrc[:, :], 0)
            warm_ps = psc.tile([128, NB], mybir.dt.float32, tag="convps")
            for _ in range(WARM_MMS):
                nc.tensor.matmul(
                    warm_ps[:, :], wsrc[:, :128], wsrc[:, :],
                    start=True, stop=True,
                )
            # Zero all 8 psum banks once: ReLU reads [0:116] across the
            # gap partitions 52..63 (FC weight rows there are 0, but
            # relu(NaN-garbage) would still poison the FC), and the Sel
            # matmul reads unwritten rows of the FC partial bank.
            for _ in range(6):
                t = psc.tile([128, NB], mybir.dt.float32, tag="convps",
                             name="gapz")
                nc.vector.memset(t[:, :], 0)
            for _ in range(2):
                t = pso.tile([128, NB], mybir.dt.float32, tag="outps",
                             name="gapzo")
                nc.vector.memset(t[:, :], 0)

            xts = {}

            def issue_x(j):
                if j >= N_BLOCKS or j in xts:
                    return
                xt = const.tile([XPART, N_PASS, NB], bf, name=f"xb{j}")
                nc.sync.dma_start(out=xt[0:56, :, :],
                                  in_=xP[0:56, j, :, :])
                nc.gpsimd.dma_start(out=xt[64:120, :, :],
                                    in_=xP[64:120, j, :, :])
                xts[j] = xt

            state = {}

            def emit_conv(j):
                xt = xts[j]
                pairs = {}
                written = {}

                def pt(g):
                    k = g // 2
                    if k not in pairs:
                        # pair 6 (group 12 alone) ping-pongs with the FC
                        # partial bank in the pso pool: keeps the six psc
                        # banks on a stable pair->bank mapping.
                        pool = pso if k == 6 else psc
                        tag = "outps" if k == 6 else "convps"
                        pairs[k] = pool.tile([128, NB], mybir.dt.float32,
                                             tag=tag, name=f"pair{k}")
                    s = 64 * (g % 2)
                    return pairs[k][s:s + G2, :]

                def mm(g, stat, rb):
                    first = g not in written
                    written[g] = True
                    nc.tensor.matmul(
                        pt(g), stat(rb), xt[rb:rb + 56, p, :],
                        start=first, stop=not first,
                        tile_position=(rb, 64 * (g % 2)),
                        skip_group_check=True,
                    )

                feats = {}
                for p in range(N_PASS):
                    mm(p, SA, 0)                    # A: chunk p -> group p
                    if p >= 1:
                        mm(p - 1, SB, 0)            # B: chunk p -> group p-1
                    if p <= 5:
                        mm(p + 7, SA, 64)           # A: chunk p+7 -> group p+7
                    mm(p + 6, SB, 64)               # B: chunk p+7 -> group p+6
                    for k in RELU_AFTER.get(p, ()):
                        kf = KFC if k < N_FC - 1 else G2
                        ft = fpool.tile([kf, NB], bf, tag="feat", name=f"ft{k}")
                        if k in RELU_ON_ACT:
                            nc.scalar.activation(
                                ft[:, :], pairs[k][:kf, :], Relu)
                        else:
                            nc.vector.tensor_scalar_max(
                                ft[:, :], pairs[k][:kf, :], 0.0)
                        feats[k] = ft
                state[j] = feats

            def emit_fc(j):
                feats = state.pop(j)
                # 7 col-tiled matmuls into one PSUM bank: round 1 strips
                # 0..3 (each clears its strip), round 2 strips 0..2 accum.
                ops = pso.tile([128, NB], mybir.dt.float32, tag="outps")
                for t in range(N_FC):
                    kf = KFC if t < N_FC - 1 else G2
                    strip = FCM * (t % 4)
                    nc.tensor.matmul(
                        ops[strip:strip + FCM, :], WP(t, kf),
                        feats[t][:, :],
                        start=(t < 4), stop=(t >= 3),
                        tile_position=(0, strip), skip_group_check=True,
                    )
                fcsb = fcpool.tile([128, NB], bf, tag="fcsb")
                nc.vector.tensor_copy(fcsb[:, :], ops[:, :])
                # Sel output reuses the fc-partial bank (already copied out).
                nc.tensor.matmul(
                    ops[:NOUT, :], sel[:, :], fcsb[:, :], start=True,
                    stop=True, skip_group_check=True,
                )
                osb = opool.tile([NOUT, NB], f32, tag="osb")
                nc.vector.tensor_scalar(
                    osb[:, :], ops[:NOUT, :], bias_sb[:, :], None,
                    op0=mybir.AluOpType.add,
                )
                osbs[j] = osb

            osbs = {}
            issue_x(0)
            for j in range(N_BLOCKS):
                emit_conv(j)
                issue_x(j + 1)
                if j >= 1:
                    emit_fc(j - 1)
            emit_fc(N_BLOCKS - 1)
            # Output stores last, on the sync sequencer (its x descgens all
            # run early): each store's descriptor-gen waits only its own
            # bias result, so stores pace with compute and never head-of-
            # line-block the relu stream or pollute x DMA-lane thresholds.
            for j in range(N_BLOCKS):
                nc.sync.dma_start(out=outT[:, j, :], in_=osbs[j][:, :])

    nc.finalize()
    return nc


def prepare_inputs(x, conv_w, W, b):
    SA, SB = build_conv_mats(conv_w)

    blob = np.zeros((128, BLOB_COLS), np.float32)
    blob[0:56, OFF_SA:OFF_SA + G2] = SA
    blob[64:120, OFF_SA:OFF_SA + G2] = SA
    blob[0:56, OFF_SB:OFF_SB + G2] = SB
    blob[64:120, OFF_SB:OFF_SB + G2] = SB

    Wf = np.asarray(W, np.float32)
    for t in range(N_FC):
        c0 = OFF_WP + FCM * t
        blob[0:G2, c0:c0 + NOUT] = Wf[G2 * 2 * t:G2 * (2 * t + 1), :]
        if t < N_FC - 1:
            blob[64:64 + G2, c0:c0 + NOUT] = Wf[G2 * (2 * t + 1):
                                                G2 * (2 * t + 2), :]
    blob[:, OFF_SEL:OFF_SEL + NOUT] = build_selector()
    blob = blob.astype(BF16)

    bias = np.asarray(b, np.float32).reshape(NOUT, 1)

    # Pack x: [B, 784] -> per core [120, N_BLOCKS, 7, NB] bf16
    # (partition-major across blocks for contiguous super-chunk DMAs).
    xbf = np.asarray(x, np.float32).astype(BF16)
    # [core, block, b, row, col] view of the batch-major input
    xv = xbf.reshape(N_CORES, N_BLOCKS, NB, IMG, IMG)
    in_maps = []
    for core in range(N_CORES):
        xp = np.zeros((XPART, N_BLOCKS, N_PASS, NB), BF16)
        for c in range(N_PASS):
            for r in range(2):
                # lo: chunk c rows 2c+r; hi: chunk c+7 rows 2c+14+r
                xp[r * IMG:(r + 1) * IMG, :, c, :] = (
                    xv[core, :, :, 2 * c + r, :].transpose(2, 0, 1)
                )
                xp[64 + r * IMG:64 + (r + 1) * IMG, :, c, :] = (
                    xv[core, :, :, 2 * c + 14 + r, :].transpose(2, 0, 1)
                )
        in_maps.append({"xP": xp, "blob": blob, "bias": bias})
    return in_maps


def run(x, conv_w, W, b, trace=False, **spmd_kwargs):
    in_maps = prepare_inputs(x, conv_w, W, b)
    nc = build_program()
    res = run_bass_kernel_spmd(
        nc, in_maps, list(range(N_CORES)), trace=trace, **spmd_kwargs
    )
    out = np.empty((B_FULL, NOUT), np.float32)
    for c in range(N_CORES):
        out[c * B_CORE:(c + 1) * B_CORE, :] = (
            res.results[c]["outT"].reshape(NOUT, B_CORE).T
        )
    return out, res


def kernel(x, conv_w, W, b):
    out, _ = run(x, conv_w, W, b, trace=False)
    return out
